# revision 13
# baseline (speedup 1.0000x reference)
"""Trainium2 Bass kernel for nn_DifferentiableRobotModel (self-collision link
distances from batched forward kinematics).

Pure data parallel over the batch (rollout) dim: 8192 rollouts -> 1024/core
on 8 NeuronCores. All FK params / sphere tables / masks are tiny and
replicated.

v3 pipeline (per core, bc = 1024 batches, 128 spheres = 16 links x 8),
"quantized min-first" formulation:
  * Spheres are globally relabeled in r-sorted order (slot p = sphere
    pi[p]); 16 r-levels of 8 spheres each get a representative rbar_g.
    max_j (r_j - d_ij) ~= max_g (rbar_g - sqrt(min_{j in g} d2_ij)), so the
    expensive per-pair sqrt / subtract / mask passes collapse into one
    segmented-min over raw d2 straight out of PSUM.
  * The pair mask (|link_i - link_j| <= 1 band) is folded INTO the gram
    matmul: lhsT rows 112..127 hold BIG*[link(i)==m], rhs rows 112..127
    hold the band indicator [|m - link(j)| <= 1]; masked pairs get d2+BIG
    and never win the min. Zero extra PE cycles (matmul cost = columns).
  1. FK on DVE, batch-on-partitions (serial chain over 16 links using
     M_l(q) = P_l + sin(q) Q_l + (1-cos q) S_l, host-precomputed P/Q/S),
     recentered by t_link8.
  2. PE transposes S planes -> mall; world centers via bd2xp^T @ mall
     (bd2xp columns in slot order); derived ctt1/ctt2 attr planes with
     sqh/sql split of |c|^2 for fp32-grade d2 accuracy.
  3. Per 16-batch chunk: 2 PE transposes -> tta ring [128,256] (rows
     0..111 dynamic, 112..127 static mask); 4 batched scatter DMAs per
     128-batch tile build the block-diagonal X operand.
  4. Grams: 512-col fp16 matmuls -> d2' [128 i, 8 batch * 128 j] in PSUM.
  5. Per gram tile, two routes: (A) DVE segmented min (seg 8) PSUM->fp16,
     (B) ACT cast PSUM->fp16 + GpSimd 3-level pairwise-min tree.
     Both land m2[i, (b,g)] per tile.
  6. Stage2 per tile [128, 2048]: ACT sqrt(+eps), DVE sub rbar_g, DVE
     segmented max over g -> z[i,b].
  7. Tail: z + r_i, PE transpose with the inverse permutation matrix
     (un-relabels spheres for free), grouped max over each link's 8
     spheres -> out [batch, 16].
"""
import sys
import numpy as np

sys.path.insert(0, "/opt/trn_rl_repo")

import concourse.bass as bass  # noqa: E402
import concourse.tile as tile  # noqa: E402
from concourse import bacc, mybir  # noqa: E402
from contextlib import ExitStack  # noqa: E402

F32 = mybir.dt.float32
F16 = mybir.dt.float16
AF = mybir.ActivationFunctionType
ALU = mybir.AluOpType
AX = mybir.AxisListType

B, L, NS = 8192, 16, 8
N = L * NS              # 128 spheres
NCORES = 8
G = 16                  # r-quantization levels (8 spheres each, r-sorted)
SEG = N // G            # spheres per level
EPS = np.float32(4e-6)  # d2 positivity shift
BIG = np.float32(60.0)  # mask offset added to d2 of ignored pairs
CHUNK = 16              # batches per transpose/gram chunk (7*16=112 rows)
NB_ROUTE = 5            # of every 8 gram tiles, this many take the ACT route

_CACHE = {}


# ---------------------------------------------------------------- host consts
def _host_consts(fixed_rot, fixed_trans, joint_axes, link_spheres,
                 collision_mask, bc):
    f32, f16 = np.float32, np.float16
    ax = np.asarray(joint_axes, f32)
    K = np.zeros((L, 3, 3), f32)
    K[:, 0, 1], K[:, 0, 2] = -ax[:, 2], ax[:, 1]
    K[:, 1, 0], K[:, 1, 2] = ax[:, 2], -ax[:, 0]
    K[:, 2, 0], K[:, 2, 1] = -ax[:, 1], ax[:, 0]
    K2 = np.einsum("lij,ljk->lik", K, K).astype(f32)
    A = np.asarray(fixed_rot, f32)
    P = A
    Q = np.einsum("lij,ljk->lik", A, K).astype(f32)
    S = np.einsum("lij,ljk->lik", A, K2).astype(f32)

    # pqs [128, 576]: sections P/Q/S as 3x4 blocks, col sec*192 + l*12 +
    # j*4 + b4; b4==3 holds ftrans (P section) / 0 (Q,S sections)
    ft = np.asarray(fixed_trans, f32)
    pqs = np.zeros((128, 576), f32)
    for sec, Mx in enumerate((P, Q, S)):
        blk = np.zeros((L, 3, 4), f32)
        blk[:, :, :3] = Mx
        if sec == 0:
            blk[:, :, 3] = ft
        pqs[:, sec * 192:(sec + 1) * 192] = blk.reshape(1, L * 12)

    x = np.asarray(link_spheres, f32)[..., :3]           # [L,NS,3]
    r = np.asarray(link_spheres, f32)[..., 3].reshape(N)

    # slot relabeling: slot p holds original sphere pi[p], r ascending
    pi = np.argsort(r, kind="stable")
    rs = r[pi]                                           # r per slot
    lnk = pi // NS                                       # link per slot
    rbar_g = np.empty(G, f32)
    for g in range(G):
        seg = rs[SEG * g:SEG * (g + 1)]
        rbar_g[g] = 0.5 * (seg.min() + seg.max())

    # bd2xp [64, 128] fp16: rows (l, m): slot cols; col p active on rows of
    # link lnk[p]
    bd2xp = np.zeros((64, N), f32)
    for p in range(N):
        j = pi[p]
        l = j // NS
        bd2xp[4 * l:4 * l + 3, p] = x[l, j % NS, :]
        bd2xp[4 * l + 3, p] = 1.0
    bd2xp = bd2xp.astype(f16)

    # mask tables (slot space). allowed = band AND collision_mask input.
    # The gram computes NEGATED distances (-d2); masked pairs get -BIG so
    # they lose every max. xbd mask rows hold -1, tta rows hold +BIG.
    cm = np.asarray(collision_mask)
    bandmask = np.zeros((L, N), f32)                     # [m, slot j]
    for m in range(L):
        for p in range(N):
            lj = lnk[p]
            ignored = (abs(m - lj) <= 1) or (not cm[m, lj])
            bandmask[m, p] = -1.0 if ignored else 0.0
    # xbdm [16, 16384]: band pattern tiled over 8 chunks * 16 batches
    xbdm = np.tile(bandmask, (1, 128)).astype(f16)
    # ttam [16, 4096]: 16 slots of 256; first 128 cols = BIG*[lnk==m]
    tslot = np.zeros((L, 256), f32)
    for m in range(L):
        tslot[m, :N] = BIG * (lnk == m)
    ttam = np.tile(tslot, (1, 16)).astype(f16)

    # rbar16 [128, G], rcolp [128, 1] (r per slot)
    rbar16 = np.tile(rbar_g.reshape(1, G), (128, 1)).astype(f16)
    rcolp = rs.reshape(N, 1).astype(f32)
    # tail un-permutation: ztr = z2^T @ p16; p16[p, c] = [pi[p] == c]
    p16 = np.zeros((N, N), f16)
    p16[np.arange(N), pi] = 1.0

    ident = np.eye(128, dtype=f32)
    ident16 = np.eye(128, dtype=f16)
    epsb = np.full((128, 1), EPS, f32)
    return dict(pqs=pqs, bd2xp=bd2xp, xbdm=xbdm, ttam=ttam,
                rbar16=rbar16, rcolp=rcolp, p16=p16,
                ident=ident, ident16=ident16, epsb=epsb)


# ---------------------------------------------------------------- device build
def _build_nc(nt):
    """Build + compile the per-core Bass module for nt tiles of 128 batches."""
    bc = nt * 128
    nc = bacc.Bacc("TRN2", target_bir_lowering=False, debug=False,
                   num_devices=NCORES)

    q_d = nc.dram_tensor("q", [bc, L], F32, kind="ExternalInput").ap()
    pqs_d = nc.dram_tensor("pqs", [128, 576], F32, kind="ExternalInput").ap()
    bd2xp_d = nc.dram_tensor("bd2xp", [64, N], F16, kind="ExternalInput").ap()
    xbdm_d = nc.dram_tensor("xbdm", [16, 16384], F16,
                            kind="ExternalInput").ap()
    ttam_d = nc.dram_tensor("ttam", [16, 4096], F16, kind="ExternalInput").ap()
    rbar_d = nc.dram_tensor("rbar16", [128, G], F16, kind="ExternalInput").ap()
    rcolp_d = nc.dram_tensor("rcolp", [N, 1], F32, kind="ExternalInput").ap()
    p16_d = nc.dram_tensor("p16", [N, N], F16, kind="ExternalInput").ap()
    ident_d = nc.dram_tensor("ident", [128, 128], F32,
                             kind="ExternalInput").ap()
    ident16_d = nc.dram_tensor("ident16", [128, 128], F16,
                               kind="ExternalInput").ap()
    epsb_d = nc.dram_tensor("epsb", [128, 1], F32, kind="ExternalInput").ap()
    out_d = nc.dram_tensor("out", [bc, L], F32, kind="ExternalOutput").ap()

    # persistent SBUF tensors
    qsb = nc.alloc_sbuf_tensor("qsb", [128, 16 * nt], F32).ap()
    sinb = nc.alloc_sbuf_tensor("sinb", [128, 16 * nt], F32).ap()
    cosb = nc.alloc_sbuf_tensor("cosb", [128, 16 * nt], F32).ap()
    omcb = nc.alloc_sbuf_tensor("omcb", [128, 16 * nt], F32).ap()
    pqs = nc.alloc_sbuf_tensor("pqs_sb", [128, 576], F32).ap()
    bd2xp = nc.alloc_sbuf_tensor("bd2xp_sb", [64, N], F16).ap()
    rbar = nc.alloc_sbuf_tensor("rbar_sb", [128, G], F16).ap()
    rcolp = nc.alloc_sbuf_tensor("rcolp_sb", [N, 1], F32).ap()
    p16 = nc.alloc_sbuf_tensor("p16_sb", [128, 128], F16).ap()
    ident = nc.alloc_sbuf_tensor("ident_sb", [128, 128], F32).ap()
    ident16 = nc.alloc_sbuf_tensor("ident16_sb", [128, 128], F16).ap()
    epsb = nc.alloc_sbuf_tensor("epsb_sb", [128, 1], F32).ap()
    # FK state, homogeneous 3x4, plane-major: col = t*204 + a*68 + slot*4
    # + b4 holds H[a,b4] = [R | t]; slot 0 = identity pose. Plane-major so
    # the S-plane transpose read (slot, b4) collapses to one contiguous run.
    SP = 204
    sfk = nc.alloc_sbuf_tensor("sfk", [128, SP * nt], F32).ap()
    mw = nc.alloc_sbuf_tensor("mw", [128, 192 * nt], F32).ap()
    mw2 = nc.alloc_sbuf_tensor("mw2", [128, 192 * nt], F32).ap()
    tscr = nc.alloc_sbuf_tensor("tscr", [128, 12 * nt], F32).ap()
    # mall [64, 3*bc] fp16, batch-major per tile: col = b*3 + k
    mall = nc.alloc_sbuf_tensor("mall", [64, 3 * bc], F16).ap()
    # ctt1/ctt2 [128, 7*bc] fp16: col = b*7 + attr (contiguous per batch)
    # ctt1 (T1): 0-2 c, 3/4 one, 5 sqh, 6 sql
    # ctt2 (T2): 0-2 -2c, 3 sqh, 4 sql, 5/6 one
    ctt1 = nc.alloc_sbuf_tensor("ctt1", [128, 7 * bc], F16).ap()
    ctt2 = nc.alloc_sbuf_tensor("ctt2", [128, 7 * bc], F16).ap()
    # tta ring: 16 slots of [128, 256] (T1|T2 per chunk); rows 112..127
    # static mask rows (DMA'd once); halves alternate per tile
    tta = nc.alloc_sbuf_tensor("tta", [128, 16 * 256], F16).ap()
    # block-diag gram moving operands: per tile [128, 8*2048]
    # rows 0..111 zeros + scatter-DMA'd diagonal; rows 112..127 static band
    xbd0 = nc.alloc_sbuf_tensor("xbd0", [128, 16384], F16).ap()
    xbd1 = nc.alloc_sbuf_tensor("xbd1", [128, 16384], F16).ap()

    def cap(base, offset, dims):
        """Custom AP on a persistent tensor: dims = [[step,count],...] (free)."""
        pitch = base.tensor.shape[-1]
        nparts = base.tensor.shape[0]
        return bass.AP(tensor=base.tensor, offset=offset,
                       ap=[[pitch, nparts]] + list(dims))

    def capp(base, prow, nrow, offset, dims):
        """Custom AP with partition sub-range [prow, prow+nrow)."""
        pitch = base.tensor.shape[-1]
        return bass.AP(tensor=base.tensor, offset=prow * pitch + offset,
                       ap=[[pitch, nrow]] + list(dims))

    def tap(tl, off, dims):
        """Custom free-dim AP on a pool tile (keeps its partition dim)."""
        a = tl[:, :]
        return bass.AP(tensor=a.tensor, offset=a.offset + off,
                       ap=[list(a.ap[0])] + list(dims))

    with tile.TileContext(nc) as tc, ExitStack() as ctx:
        prepool = ctx.enter_context(tc.tile_pool(name="pre", bufs=1,
                                                 space="PSUM"))
        ttpool = ctx.enter_context(tc.tile_pool(name="ttp", bufs=2,
                                                space="PSUM"))
        grpool = ctx.enter_context(tc.tile_pool(name="gram", bufs=2,
                                                space="PSUM"))
        sqwp = ctx.enter_context(tc.tile_pool(name="sqw", bufs=2))
        ycp = ctx.enter_context(tc.tile_pool(name="yc", bufs=3))
        t1p = ctx.enter_context(tc.tile_pool(name="t1", bufs=2))
        t2p = ctx.enter_context(tc.tile_pool(name="t2", bufs=2))
        mmp = ctx.enter_context(tc.tile_pool(name="mm", bufs=2))
        s2p = ctx.enter_context(tc.tile_pool(name="s2", bufs=2))
        y2p = ctx.enter_context(tc.tile_pool(name="y2", bufs=2))
        zpool = ctx.enter_context(tc.tile_pool(name="z", bufs=2))
        z2pool = ctx.enter_context(tc.tile_pool(name="z2", bufs=2))
        outp = ctx.enter_context(tc.tile_pool(name="outsb", bufs=2))

        # ---- input DMAs
        nc.sync.dma_start(pqs, pqs_d)
        nc.sync.dma_start(bd2xp, bd2xp_d)
        nc.sync.dma_start(rbar, rbar_d)
        nc.sync.dma_start(rcolp, rcolp_d)
        nc.sync.dma_start(p16, p16_d)
        nc.sync.dma_start(ident, ident_d)
        nc.sync.dma_start(ident16, ident16_d)
        nc.sync.dma_start(epsb, epsb_d)
        # static mask rows 112..127 of tta / xbd0 / xbd1
        nc.sync.dma_start(capp(tta, 112, 16, 0, [[1, 4096]]), ttam_d)
        nc.sync.dma_start(capp(xbd0, 112, 16, 0, [[1, 16384]]), xbdm_d)
        nc.sync.dma_start(capp(xbd1, 112, 16, 0, [[1, 16384]]), xbdm_d)
        for t in range(nt):
            nc.sync.dma_start(cap(qsb, 16 * t, [[1, 16]]),
                              q_d[128 * t:128 * (t + 1), :])

        # ---- sin / cos / (1-cos)
        nc.scalar.activation(sinb, qsb, AF.Sin)
        # 1 - cos(q) = 2 sin^2(q/2); Sin LUT domain is [-pi, pi]
        nc.scalar.activation(cosb, qsb, AF.Sin, scale=0.5)
        nc.vector.tensor_mul(omcb, cosb, cosb)
        nc.vector.tensor_scalar_mul(omcb, omcb, 2.0)

        # ---- zero-fill: slot0 of sfk = identity pose [I | 0]
        nc.vector.memset(cap(sfk, 0, [[SP, nt], [68, 3], [1, 4]]), 0.0)
        nc.vector.memset(cap(sfk, 0, [[SP, nt], [69, 3]]), 1.0)  # I diag
        # const-1 planes
        nc.vector.memset(cap(ctt1, 3, [[7, bc], [1, 2]]), 1.0)
        nc.vector.memset(cap(ctt2, 5, [[7, bc], [1, 2]]), -1.0)
        # block-diag X dynamic rows zeros (written once; only diagonal blocks
        # rewritten by scatter DMAs)
        nc.gpsimd.memset(capp(xbd0, 0, 112, 0, [[1, 16384]]), 0.0)
        nc.vector.memset(capp(xbd1, 0, 112, 0, [[1, 16384]]), 0.0)

        # ---- [M_l | f_l] = P4 + sin*Q4 + (1-cos)*S4: mw[(t,l,(j,b4))]
        mdims = [[192, nt], [12, L], [1, 12]]
        sdims = [[16, nt], [1, L], [0, 12]]
        nc.gpsimd.tensor_tensor(cap(mw, 0, mdims),
                                cap(pqs, 192, [[0, nt]] + mdims[1:]),
                                cap(sinb, 0, sdims), ALU.mult)
        nc.gpsimd.tensor_tensor(cap(mw2, 0, mdims),
                                cap(pqs, 384, [[0, nt]] + mdims[1:]),
                                cap(omcb, 0, sdims), ALU.mult)
        nc.vector.tensor_add(mw, mw, mw2)
        nc.vector.tensor_add(cap(mw, 0, mdims), cap(mw, 0, mdims),
                             cap(pqs, 0, [[0, nt]] + mdims[1:]))

        # ---- FK serial chain: H_l = Hp @ [M_l|f_l] + [0|tp]
        for l in range(L):
            sp, s_ = 4 * l, 4 * (l + 1)       # prev slot, this slot (col/4)
            outH = cap(sfk, s_, [[SP, nt], [68, 3], [1, 4]])
            tmpH = cap(tscr, 0, [[12, nt], [4, 3], [1, 4]])
            for j in range(3):
                i0 = cap(sfk, sp + j, [[SP, nt], [68, 3], [0, 4]])
                i1 = cap(mw, 12 * l + 4 * j, [[192, nt], [0, 3], [1, 4]])
                if j == 0:
                    nc.vector.tensor_mul(outH, i0, i1)
                else:
                    nc.vector.tensor_mul(tmpH, i0, i1)
                    nc.vector.tensor_add(outH, outH, tmpH)
            # t_l += t_p
            nc.vector.tensor_add(cap(sfk, s_ + 3, [[SP, nt], [68, 3]]),
                                 cap(sfk, s_ + 3, [[SP, nt], [68, 3]]),
                                 cap(sfk, sp + 3, [[SP, nt], [68, 3]]))

        # ---- recenter: t'_l = t_l - t_link8 (slot 9)
        nc.vector.tensor_copy(cap(tscr, 0, [[12, nt], [1, 3]]),
                              cap(sfk, 4 * 9 + 3, [[SP, nt], [68, 3]]))
        nc.vector.tensor_sub(cap(sfk, 4 + 3, [[SP, nt], [68, 3], [4, L]]),
                             cap(sfk, 4 + 3, [[SP, nt], [68, 3], [4, L]]),
                             cap(tscr, 0, [[12, nt], [1, 3], [0, L]]))

        # ---- per tile: S planes k=0..2 -> mall; CT matmul -> ctt planes 0..2
        for t in range(nt):
            trm = prepool.tile([64, 384], F32, tag="pre")
            for k in range(3):
                nc.tensor.transpose(
                    trm[:, 128 * k:128 * (k + 1)],
                    cap(sfk, SP * t + 68 * k + 4, [[1, 64]]),
                    ident)
            # mall batch-major: col = b*3 + k  (trm col = k*128 + b)
            nc.scalar.copy(
                capp(mall, 0, 64, 3 * 128 * t, [[1, 3], [3, 128]]),
                trm[:, :])
            # world centers: ctp[j, (b,k)] = bd2xp^T @ mall
            ctp = prepool.tile([128, 384], F32, tag="pre")
            nc.tensor.matmul(
                ctp[:, :],
                bd2xp[0:64, :],
                capp(mall, 0, 64, 3 * 128 * t, [[1, 384]]))
            nc.scalar.copy(
                cap(ctt1, 7 * 128 * t, [[7, 128], [1, 3]]),
                ctp[:, :])
            # derived planes
            c_ap = cap(ctt1, 7 * 128 * t, [[7, 128], [1, 3]])
            sqw = sqwp.tile([128, 384], F32)
            nc.gpsimd.tensor_tensor(sqw[:, :], c_ap, c_ap, ALU.mult)
            sq32 = sqwp.tile([128, 128], F32)
            nc.vector.reduce_sum(
                sq32[:, :], sqw[:, :].rearrange("p (b k) -> p b k", k=3),
                axis=AX.X)
            # sqh (fp16) and sql = sq - sqh
            nc.gpsimd.tensor_copy(cap(ctt1, 7 * 128 * t + 5, [[7, 128]]),
                                  sq32[:, :])
            nc.vector.tensor_sub(cap(ctt1, 7 * 128 * t + 6, [[7, 128]]),
                                 sq32[:, :],
                                 cap(ctt1, 7 * 128 * t + 5, [[7, 128]]))
            nc.vector.tensor_scalar_mul(
                cap(ctt2, 7 * 128 * t + 3, [[7, 128], [1, 2]]),
                cap(ctt1, 7 * 128 * t + 5, [[7, 128], [1, 2]]), -1.0)
            nc.vector.tensor_scalar_mul(
                cap(ctt2, 7 * 128 * t + 0, [[7, 128], [1, 3]]),
                cap(ctt1, 7 * 128 * t + 0, [[7, 128], [1, 3]]), 2.0)

        # ---- main loop: tiles of 128 batches (8 chunks of 16)
        gri_all = 0
        for t in range(nt):
            half = t % 2
            # transposes into the tta ring half
            for cc in range(8):
                c = 8 * t + cc
                slot = 256 * (8 * half + cc)
                tt = ttpool.tile([112, 256], F16)
                nc.tensor.transpose(
                    tt[:, 0:128],
                    cap(ctt1, 7 * CHUNK * c, [[1, 112]]),
                    ident16)
                nc.tensor.transpose(
                    tt[:, 128:256],
                    cap(ctt2, 7 * CHUNK * c, [[1, 112]]),
                    ident16)
                nc.scalar.copy(capp(tta, 0, 112, slot, [[1, 256]]), tt[:, :])
            # batched scatter: 4 DMAs, each moves 4 k-blocks of all 8 chunks
            xbd = xbd0 if half == 0 else xbd1
            tpitch = tta.tensor.shape[-1]
            xpitch = xbd.tensor.shape[-1]
            for k in range(CHUNK):
                src_ap = bass.AP(
                    tensor=tta.tensor,
                    offset=7 * k * tpitch + 256 * 8 * half + 128,
                    ap=[[tpitch, 7], [256, 8], [1, 128]])
                dst_ap = bass.AP(
                    tensor=xbd.tensor,
                    offset=7 * k * xpitch + 128 * k,
                    ap=[[xpitch, 7], [2048, 8], [1, 128]])
                eng = nc.sync if k % 2 == 0 else nc.scalar
                eng.dma_start(dst_ap, src_ap)
            # grams + min-reduce per chunk
            mm = mmp.tile([128, 16 * G * 8], F16)  # [128, 2048]
            for cc in range(8):
                slot = 256 * (8 * half + cc)
                for h in range(2):
                    gri = gri_all
                    gri_all += 1
                    gr = grpool.tile([128, 1024], F32)
                    for g2 in range(2):
                        nc.tensor.matmul(
                            gr[:, 512 * g2:512 * (g2 + 1)],
                            capp(tta, 0, 128, slot, [[1, 128]]),
                            capp(xbd, 0, 128,
                                 2048 * cc + 512 * (2 * h + g2), [[1, 512]]))
                    m2 = mm[:, 128 * (2 * cc + h):128 * (2 * cc + h) + 128]
                    if gri % 4 != 0:
                        # route B: ACT cast (de-interleaved so the min tree
                        # is 3 contiguous-half TTs on DVE) + DVE max tree.
                        # gr col = b*128 + g*8 + u -> yc col = u*128 + b*16 + g
                        yc = ycp.tile([128, 1024], F16)
                        nc.scalar.copy(
                            tap(yc, 0, [[16, 8], [1, 16], [128, 8]]),
                            tap(gr, 0, [[128, 8], [8, 16], [1, 8]]))
                        w1 = t1p.tile([128, 512], F16)
                        nc.vector.tensor_tensor(
                            w1[:, :], yc[:, 0:512], yc[:, 512:1024], ALU.max)
                        w2 = t2p.tile([128, 256], F16)
                        nc.vector.tensor_tensor(
                            w2[:, :], w1[:, 0:256], w1[:, 256:512], ALU.max)
                        nc.vector.tensor_tensor(
                            m2, w2[:, 0:128], w2[:, 128:256], ALU.max)
                    else:
                        # route A: DVE segmented max straight out of PSUM
                        nc.vector.tensor_reduce(
                            m2,
                            gr[:, :].rearrange("p (s j) -> p s j", j=SEG),
                            axis=AX.X, op=ALU.max)
            # stage2: sqrt, sub rbar, segmented max over levels
            s2 = s2p.tile([128, 2048], F16)
            nc.scalar.activation(s2[:, :], mm[:, :], AF.Sqrt,
                                 bias=epsb[0:128, 0:1], scale=-1.0)
            y2 = y2p.tile([128, 2048], F16)
            nc.gpsimd.tensor_tensor(
                y2[:, :].rearrange("p (b g) -> p b g", g=G),
                cap(rbar, 0, [[0, 128], [1, G]]),
                s2[:, :].rearrange("p (b g) -> p b g", g=G), ALU.subtract)
            z = zpool.tile([128, 128], F16)
            nc.vector.tensor_reduce(
                z[:, :], y2[:, :].rearrange("p (b g) -> p b g", g=G),
                axis=AX.X, op=ALU.max)
            # tail for this tile
            z2 = z2pool.tile([128, 128], F16)
            nc.vector.tensor_scalar_add(z2[:, :], z[:, :],
                                        rcolp[0:128, 0:1])
            ztr = prepool.tile([128, 128], F16, tag="pre")
            nc.tensor.transpose(ztr[:, :], z2[:, :], p16[0:128, 0:128])
            osb = outp.tile([128, L], F32)
            nc.vector.tensor_reduce(
                osb[:, :], ztr[:, :].rearrange("p (a b) -> p a b", a=L),
                axis=AX.X, op=ALU.max)
            nc.sync.dma_start(out_d[128 * t:128 * (t + 1), :], osb[:, :])

    nc.compile()
    return nc


def get_nc(nt):
    key = ("nc", nt)
    if key not in _CACHE:
        _CACHE[key] = _build_nc(nt)
    return _CACHE[key]


# ---------------------------------------------------------------- entry point
def kernel(q, fixed_rot, fixed_trans, joint_axes, link_spheres,
           collision_mask):
    from concourse.bass_utils import run_bass_kernel_spmd

    q = np.asarray(q, np.float32)
    bc = B // NCORES
    nt = bc // 128
    consts = _host_consts(fixed_rot, fixed_trans, joint_axes, link_spheres,
                          collision_mask, bc)
    nc = get_nc(nt)
    in_maps = []
    for c in range(NCORES):
        m = {"q": np.ascontiguousarray(q[c * bc:(c + 1) * bc]), **consts}
        in_maps.append(m)
    res = run_bass_kernel_spmd(nc, in_maps, list(range(NCORES)))
    out = np.concatenate([res.results[c]["out"] for c in range(NCORES)],
                         axis=0)
    return out.astype(np.float32)


# revision 14
# speedup vs baseline: 1.5961x; 1.5961x over previous
"""Trainium2 Bass kernel for nn_DifferentiableRobotModel (self-collision link
distances from batched forward kinematics).

Pure data parallel over the batch (rollout) dim: 8192 rollouts -> 1024/core
on 8 NeuronCores. All FK params / sphere tables / masks are tiny and
replicated.

v3 pipeline (per core, bc = 1024 batches, 128 spheres = 16 links x 8),
"quantized min-first" formulation:
  * Spheres are globally relabeled in r-sorted order (slot p = sphere
    pi[p]); 16 r-levels of 8 spheres each get a representative rbar_g.
    max_j (r_j - d_ij) ~= max_g (rbar_g - sqrt(min_{j in g} d2_ij)), so the
    expensive per-pair sqrt / subtract / mask passes collapse into one
    segmented-min over raw d2 straight out of PSUM.
  * The pair mask (|link_i - link_j| <= 1 band) is folded INTO the gram
    matmul: lhsT rows 112..127 hold BIG*[link(i)==m], rhs rows 112..127
    hold the band indicator [|m - link(j)| <= 1]; masked pairs get d2+BIG
    and never win the min. Zero extra PE cycles (matmul cost = columns).
  1. FK on DVE, batch-on-partitions (serial chain over 16 links using
     M_l(q) = P_l + sin(q) Q_l + (1-cos q) S_l, host-precomputed P/Q/S),
     recentered by t_link8.
  2. PE transposes S planes -> mall; world centers via bd2xp^T @ mall
     (bd2xp columns in slot order); derived ctt1/ctt2 attr planes with
     sqh/sql split of |c|^2 for fp32-grade d2 accuracy.
  3. Per 16-batch chunk: 2 PE transposes -> tta ring [128,256] (rows
     0..111 dynamic, 112..127 static mask); 4 batched scatter DMAs per
     128-batch tile build the block-diagonal X operand.
  4. Grams: 512-col fp16 matmuls -> d2' [128 i, 8 batch * 128 j] in PSUM.
  5. Per gram tile, two routes: (A) DVE segmented min (seg 8) PSUM->fp16,
     (B) ACT cast PSUM->fp16 + GpSimd 3-level pairwise-min tree.
     Both land m2[i, (b,g)] per tile.
  6. Stage2 per tile [128, 2048]: ACT sqrt(+eps), DVE sub rbar_g, DVE
     segmented max over g -> z[i,b].
  7. Tail: z + r_i, PE transpose with the inverse permutation matrix
     (un-relabels spheres for free), grouped max over each link's 8
     spheres -> out [batch, 16].
"""
import sys
import numpy as np

sys.path.insert(0, "/opt/trn_rl_repo")

import concourse.bass as bass  # noqa: E402
import concourse.tile as tile  # noqa: E402
from concourse import bacc, mybir  # noqa: E402
from contextlib import ExitStack  # noqa: E402

F32 = mybir.dt.float32
F16 = mybir.dt.float16
AF = mybir.ActivationFunctionType
ALU = mybir.AluOpType
AX = mybir.AxisListType

B, L, NS = 8192, 16, 8
N = L * NS              # 128 spheres
NCORES = 8
G = 16                  # r-quantization levels (8 spheres each, r-sorted)
SEG = N // G            # spheres per level
EPS = np.float32(4e-6)  # d2 positivity shift
BIG = np.float32(60.0)  # mask offset added to d2 of ignored pairs
CHUNK = 16              # batches per transpose/gram chunk (7*16=112 rows)
NB_ROUTE = 5            # of every 8 gram tiles, this many take the ACT route

_CACHE = {}


# ---------------------------------------------------------------- host consts
def _host_consts(fixed_rot, fixed_trans, joint_axes, link_spheres,
                 collision_mask, bc):
    f32, f16 = np.float32, np.float16
    ax = np.asarray(joint_axes, f32)
    K = np.zeros((L, 3, 3), f32)
    K[:, 0, 1], K[:, 0, 2] = -ax[:, 2], ax[:, 1]
    K[:, 1, 0], K[:, 1, 2] = ax[:, 2], -ax[:, 0]
    K[:, 2, 0], K[:, 2, 1] = -ax[:, 1], ax[:, 0]
    K2 = np.einsum("lij,ljk->lik", K, K).astype(f32)
    A = np.asarray(fixed_rot, f32)
    P = A
    Q = np.einsum("lij,ljk->lik", A, K).astype(f32)
    S = np.einsum("lij,ljk->lik", A, K2).astype(f32)

    # pqs [128, 576]: sections P/Q/S as 3x4 blocks, col sec*192 + l*12 +
    # j*4 + b4; b4==3 holds ftrans (P section) / 0 (Q,S sections)
    ft = np.asarray(fixed_trans, f32)
    pqs = np.zeros((128, 576), f32)
    for sec, Mx in enumerate((P, Q, S)):
        blk = np.zeros((L, 3, 4), f32)
        blk[:, :, :3] = Mx
        if sec == 0:
            blk[:, :, 3] = ft
        pqs[:, sec * 192:(sec + 1) * 192] = blk.reshape(1, L * 12)

    x = np.asarray(link_spheres, f32)[..., :3]           # [L,NS,3]
    r = np.asarray(link_spheres, f32)[..., 3].reshape(N)

    # slot relabeling: slot p holds original sphere pi[p]. Level g of slot
    # p is p % G (u-major interleave), so level members sit at stride G in
    # j and the min-tree reduces with contiguous-run access patterns.
    pr = np.argsort(r, kind="stable")                    # rank -> sphere
    pi = np.empty(N, np.int64)
    for p in range(N):
        pi[p] = pr[(p % G) * SEG + p // G]
    rs = r[pi]                                           # r per slot
    lnk = pi // NS                                       # link per slot
    rr = r[pr]
    rbar_g = np.empty(G, f32)
    for g in range(G):
        seg = rr[SEG * g:SEG * (g + 1)]
        rbar_g[g] = 0.5 * (seg.min() + seg.max())

    # bd2xp [64, 128] fp16: rows (l, m): slot cols; col p active on rows of
    # link lnk[p]
    bd2xp = np.zeros((64, N), f32)
    for p in range(N):
        j = pi[p]
        l = j // NS
        bd2xp[4 * l:4 * l + 3, p] = x[l, j % NS, :]
        bd2xp[4 * l + 3, p] = 1.0
    bd2xp = bd2xp.astype(f16)

    # mask tables (slot space). allowed = band AND collision_mask input.
    # The gram computes NEGATED distances (-d2); masked pairs get -BIG so
    # they lose every max. xbd mask rows hold -1, tta rows hold +BIG.
    cm = np.asarray(collision_mask)
    bandmask = np.zeros((L, N), f32)                     # [m, slot j]
    for m in range(L):
        for p in range(N):
            lj = lnk[p]
            ignored = (abs(m - lj) <= 1) or (not cm[m, lj])
            bandmask[m, p] = -1.0 if ignored else 0.0
    # xbdm [16, 16384]: band pattern tiled over 8 chunks * 16 batches
    xbdm = np.tile(bandmask, (1, 128)).astype(f16)
    # ttam [16, 4096]: 16 slots of 256; first 128 cols = BIG*[lnk==m]
    tslot = np.zeros((L, 256), f32)
    for m in range(L):
        tslot[m, :N] = BIG * (lnk == m)
    ttam = np.tile(tslot, (1, 16)).astype(f16)

    # rbar16 [128, G], rcolp [128, 1] (r per slot)
    rbar16 = np.tile(rbar_g.reshape(1, G), (128, 1)).astype(f16)
    rcolp = rs.reshape(N, 1).astype(f32)
    # tail un-permutation: ztr = z2^T @ p16; p16[p, c] = [pi[p] == c]
    p16 = np.zeros((N, N), f16)
    p16[np.arange(N), pi] = 1.0

    ident = np.eye(128, dtype=f32)
    ident16 = np.eye(128, dtype=f16)
    epsb = np.full((128, 1), EPS, f32)
    xzero = np.zeros((112, 16384), f16)
    return dict(pqs=pqs, bd2xp=bd2xp, xbdm=xbdm, ttam=ttam,
                rbar16=rbar16, rcolp=rcolp, p16=p16,
                ident=ident, ident16=ident16, epsb=epsb, xzero=xzero)


# ---------------------------------------------------------------- device build
def _build_nc(nt):
    """Build + compile the per-core Bass module for nt tiles of 128 batches."""
    bc = nt * 128
    nc = bacc.Bacc("TRN2", target_bir_lowering=False, debug=False,
                   num_devices=NCORES)

    q_d = nc.dram_tensor("q", [bc, L], F32, kind="ExternalInput").ap()
    pqs_d = nc.dram_tensor("pqs", [128, 576], F32, kind="ExternalInput").ap()
    bd2xp_d = nc.dram_tensor("bd2xp", [64, N], F16, kind="ExternalInput").ap()
    xbdm_d = nc.dram_tensor("xbdm", [16, 16384], F16,
                            kind="ExternalInput").ap()
    ttam_d = nc.dram_tensor("ttam", [16, 4096], F16, kind="ExternalInput").ap()
    rbar_d = nc.dram_tensor("rbar16", [128, G], F16, kind="ExternalInput").ap()
    rcolp_d = nc.dram_tensor("rcolp", [N, 1], F32, kind="ExternalInput").ap()
    p16_d = nc.dram_tensor("p16", [N, N], F16, kind="ExternalInput").ap()
    ident_d = nc.dram_tensor("ident", [128, 128], F32,
                             kind="ExternalInput").ap()
    ident16_d = nc.dram_tensor("ident16", [128, 128], F16,
                               kind="ExternalInput").ap()
    epsb_d = nc.dram_tensor("epsb", [128, 1], F32, kind="ExternalInput").ap()
    xzero_d = nc.dram_tensor("xzero", [112, 16384], F16,
                             kind="ExternalInput").ap()
    out_d = nc.dram_tensor("out", [bc, L], F32, kind="ExternalOutput").ap()

    # persistent SBUF tensors
    qsb = nc.alloc_sbuf_tensor("qsb", [128, 16 * nt], F32).ap()
    sinb = nc.alloc_sbuf_tensor("sinb", [128, 16 * nt], F32).ap()
    cosb = nc.alloc_sbuf_tensor("cosb", [128, 16 * nt], F32).ap()
    omcb = nc.alloc_sbuf_tensor("omcb", [128, 16 * nt], F32).ap()
    pqs = nc.alloc_sbuf_tensor("pqs_sb", [128, 576], F32).ap()
    bd2xp = nc.alloc_sbuf_tensor("bd2xp_sb", [64, N], F16).ap()
    rbar = nc.alloc_sbuf_tensor("rbar_sb", [128, G], F16).ap()
    rcolp = nc.alloc_sbuf_tensor("rcolp_sb", [N, 1], F32).ap()
    p16 = nc.alloc_sbuf_tensor("p16_sb", [128, 128], F16).ap()
    ident = nc.alloc_sbuf_tensor("ident_sb", [128, 128], F32).ap()
    ident16 = nc.alloc_sbuf_tensor("ident16_sb", [128, 128], F16).ap()
    epsb = nc.alloc_sbuf_tensor("epsb_sb", [128, 1], F32).ap()
    # FK state, homogeneous 3x4, plane-major: col = t*204 + a*68 + slot*4
    # + b4 holds H[a,b4] = [R | t]; slot 0 = identity pose. Plane-major so
    # the S-plane transpose read (slot, b4) collapses to one contiguous run.
    SP = 204
    sfk = nc.alloc_sbuf_tensor("sfk", [128, SP * nt], F32).ap()
    mw = nc.alloc_sbuf_tensor("mw", [128, 192 * nt], F32).ap()
    mw2 = nc.alloc_sbuf_tensor("mw2", [128, 192 * nt], F32).ap()
    tscr = nc.alloc_sbuf_tensor("tscr", [128, 12 * nt], F32).ap()
    # mall [64, 3*bc] fp16, batch-major per tile: col = b*3 + k
    mall = nc.alloc_sbuf_tensor("mall", [64, 3 * bc], F16).ap()
    # ctt1/ctt2 [128, 7*bc] fp16: col = b*7 + attr (contiguous per batch)
    # ctt1 (T1): 0-2 c, 3/4 one, 5 sqh, 6 sql
    # ctt2 (T2): 0-2 -2c, 3 sqh, 4 sql, 5/6 one
    ctt1 = nc.alloc_sbuf_tensor("ctt1", [128, 7 * bc], F16).ap()
    ctt2 = nc.alloc_sbuf_tensor("ctt2", [128, 7 * bc], F16).ap()
    # tta ring: 16 slots of [128, 256] (T1|T2 per chunk); rows 112..127
    # static mask rows (DMA'd once); halves alternate per tile
    tta = nc.alloc_sbuf_tensor("tta", [128, 16 * 256], F16).ap()
    # block-diag gram moving operands: per tile [128, 8*2048]
    # rows 0..111 zeros + scatter-DMA'd diagonal; rows 112..127 static band
    xbd0 = nc.alloc_sbuf_tensor("xbd0", [128, 16384], F16).ap()
    xbd1 = nc.alloc_sbuf_tensor("xbd1", [128, 16384], F16).ap()

    def cap(base, offset, dims):
        """Custom AP on a persistent tensor: dims = [[step,count],...] (free)."""
        pitch = base.tensor.shape[-1]
        nparts = base.tensor.shape[0]
        return bass.AP(tensor=base.tensor, offset=offset,
                       ap=[[pitch, nparts]] + list(dims))

    def capp(base, prow, nrow, offset, dims):
        """Custom AP with partition sub-range [prow, prow+nrow)."""
        pitch = base.tensor.shape[-1]
        return bass.AP(tensor=base.tensor, offset=prow * pitch + offset,
                       ap=[[pitch, nrow]] + list(dims))

    def tap(tl, off, dims):
        """Custom free-dim AP on a pool tile (keeps its partition dim)."""
        a = tl[:, :]
        return bass.AP(tensor=a.tensor, offset=a.offset + off,
                       ap=[list(a.ap[0])] + list(dims))

    with tile.TileContext(nc) as tc, ExitStack() as ctx:
        prepool = ctx.enter_context(tc.tile_pool(name="pre", bufs=1,
                                                 space="PSUM"))
        ttpool = ctx.enter_context(tc.tile_pool(name="ttp", bufs=2,
                                                space="PSUM"))
        grpool = ctx.enter_context(tc.tile_pool(name="gram", bufs=2,
                                                space="PSUM"))
        sqwp = ctx.enter_context(tc.tile_pool(name="sqw", bufs=2))
        ycp = ctx.enter_context(tc.tile_pool(name="yc", bufs=3))
        t1p = ctx.enter_context(tc.tile_pool(name="t1", bufs=2))
        t2p = ctx.enter_context(tc.tile_pool(name="t2", bufs=2))
        mmp = ctx.enter_context(tc.tile_pool(name="mm", bufs=2))
        s2p = ctx.enter_context(tc.tile_pool(name="s2", bufs=2))
        y2p = ctx.enter_context(tc.tile_pool(name="y2", bufs=2))
        zpool = ctx.enter_context(tc.tile_pool(name="z", bufs=2))
        z2pool = ctx.enter_context(tc.tile_pool(name="z2", bufs=2))
        outp = ctx.enter_context(tc.tile_pool(name="outsb", bufs=2))

        # ---- input DMAs
        nc.sync.dma_start(pqs, pqs_d)
        nc.sync.dma_start(bd2xp, bd2xp_d)
        nc.sync.dma_start(rbar, rbar_d)
        nc.sync.dma_start(rcolp, rcolp_d)
        nc.sync.dma_start(p16, p16_d)
        nc.sync.dma_start(ident, ident_d)
        nc.sync.dma_start(ident16, ident16_d)
        nc.sync.dma_start(epsb, epsb_d)
        # static mask rows 112..127 of tta / xbd0 / xbd1
        nc.sync.dma_start(capp(tta, 112, 16, 0, [[1, 4096]]), ttam_d)
        nc.sync.dma_start(capp(xbd0, 112, 16, 0, [[1, 16384]]), xbdm_d)
        nc.sync.dma_start(capp(xbd1, 112, 16, 0, [[1, 16384]]), xbdm_d)
        for t in range(nt):
            nc.sync.dma_start(cap(qsb, 16 * t, [[1, 16]]),
                              q_d[128 * t:128 * (t + 1), :])

        # ---- sin / cos / (1-cos)
        nc.scalar.activation(sinb, qsb, AF.Sin)
        # 1 - cos(q) = 2 sin^2(q/2); Sin LUT domain is [-pi, pi]
        nc.scalar.activation(cosb, qsb, AF.Sin, scale=0.5)
        nc.vector.tensor_mul(omcb, cosb, cosb)
        nc.vector.tensor_scalar_mul(omcb, omcb, 2.0)

        # ---- zero-fill: slot0 of sfk = identity pose [I | 0]
        nc.vector.memset(cap(sfk, 0, [[SP, nt], [68, 3], [1, 4]]), 0.0)
        nc.vector.memset(cap(sfk, 0, [[SP, nt], [69, 3]]), 1.0)  # I diag
        # const-1 planes
        nc.vector.memset(cap(ctt1, 3, [[7, bc], [1, 2]]), 1.0)
        nc.vector.memset(cap(ctt2, 5, [[7, bc], [1, 2]]), -1.0)
        # block-diag X dynamic rows zeros (written once; only diagonal blocks
        # rewritten by scatter DMAs)
        nc.scalar.dma_start(capp(xbd0, 0, 112, 0, [[1, 16384]]), xzero_d)
        nc.scalar.dma_start(capp(xbd1, 0, 112, 0, [[1, 16384]]), xzero_d)

        # ---- [M_l | f_l] = P4 + sin*Q4 + (1-cos)*S4: mw[(t,l,(j,b4))]
        mdims = [[192, nt], [12, L], [1, 12]]
        sdims = [[16, nt], [1, L], [0, 12]]
        nc.gpsimd.tensor_tensor(cap(mw, 0, mdims),
                                cap(pqs, 192, [[0, nt]] + mdims[1:]),
                                cap(sinb, 0, sdims), ALU.mult)
        nc.gpsimd.tensor_tensor(cap(mw2, 0, mdims),
                                cap(pqs, 384, [[0, nt]] + mdims[1:]),
                                cap(omcb, 0, sdims), ALU.mult)
        nc.vector.tensor_add(mw, mw, mw2)
        nc.vector.tensor_add(cap(mw, 0, mdims), cap(mw, 0, mdims),
                             cap(pqs, 0, [[0, nt]] + mdims[1:]))

        # ---- FK serial chain: H_l = Hp @ [M_l|f_l] + [0|tp]
        for l in range(L):
            sp, s_ = 4 * l, 4 * (l + 1)       # prev slot, this slot (col/4)
            outH = cap(sfk, s_, [[SP, nt], [68, 3], [1, 4]])
            tmpH = cap(tscr, 0, [[12, nt], [4, 3], [1, 4]])
            for j in range(3):
                i0 = cap(sfk, sp + j, [[SP, nt], [68, 3], [0, 4]])
                i1 = cap(mw, 12 * l + 4 * j, [[192, nt], [0, 3], [1, 4]])
                if j == 0:
                    nc.vector.tensor_mul(outH, i0, i1)
                else:
                    nc.vector.tensor_mul(tmpH, i0, i1)
                    nc.vector.tensor_add(outH, outH, tmpH)
            # t_l += t_p
            nc.vector.tensor_add(cap(sfk, s_ + 3, [[SP, nt], [68, 3]]),
                                 cap(sfk, s_ + 3, [[SP, nt], [68, 3]]),
                                 cap(sfk, sp + 3, [[SP, nt], [68, 3]]))

        # ---- recenter: t'_l = t_l - t_link8 (slot 9)
        nc.vector.tensor_copy(cap(tscr, 0, [[12, nt], [1, 3]]),
                              cap(sfk, 4 * 9 + 3, [[SP, nt], [68, 3]]))
        nc.vector.tensor_sub(cap(sfk, 4 + 3, [[SP, nt], [68, 3], [4, L]]),
                             cap(sfk, 4 + 3, [[SP, nt], [68, 3], [4, L]]),
                             cap(tscr, 0, [[12, nt], [1, 3], [0, L]]))

        # ---- per tile: S planes k=0..2 -> mall; CT matmul -> ctt planes 0..2
        for t in range(nt):
            trm = prepool.tile([64, 384], F32, tag="pre")
            for k in range(3):
                nc.tensor.transpose(
                    trm[:, 128 * k:128 * (k + 1)],
                    cap(sfk, SP * t + 68 * k + 4, [[1, 64]]),
                    ident)
            # mall batch-major: col = b*3 + k  (trm col = k*128 + b)
            nc.scalar.copy(
                capp(mall, 0, 64, 3 * 128 * t, [[1, 3], [3, 128]]),
                trm[:, :])
            # world centers: ctp[j, (b,k)] = bd2xp^T @ mall
            ctp = prepool.tile([128, 384], F32, tag="pre")
            nc.tensor.matmul(
                ctp[:, :],
                bd2xp[0:64, :],
                capp(mall, 0, 64, 3 * 128 * t, [[1, 384]]))
            nc.scalar.copy(
                cap(ctt1, 7 * 128 * t, [[7, 128], [1, 3]]),
                ctp[:, :])
            # derived planes
            c_ap = cap(ctt1, 7 * 128 * t, [[7, 128], [1, 3]])
            sqw = sqwp.tile([128, 384], F32)
            nc.gpsimd.tensor_tensor(sqw[:, :], c_ap, c_ap, ALU.mult)
            sq32 = sqwp.tile([128, 128], F32)
            nc.vector.reduce_sum(
                sq32[:, :], sqw[:, :].rearrange("p (b k) -> p b k", k=3),
                axis=AX.X)
            # sqh (fp16) and sql = sq - sqh
            nc.gpsimd.tensor_copy(cap(ctt1, 7 * 128 * t + 5, [[7, 128]]),
                                  sq32[:, :])
            nc.vector.tensor_sub(cap(ctt1, 7 * 128 * t + 6, [[7, 128]]),
                                 sq32[:, :],
                                 cap(ctt1, 7 * 128 * t + 5, [[7, 128]]))
            nc.vector.tensor_scalar_mul(
                cap(ctt2, 7 * 128 * t + 3, [[7, 128], [1, 2]]),
                cap(ctt1, 7 * 128 * t + 5, [[7, 128], [1, 2]]), -1.0)
            nc.vector.tensor_scalar_mul(
                cap(ctt2, 7 * 128 * t + 0, [[7, 128], [1, 3]]),
                cap(ctt1, 7 * 128 * t + 0, [[7, 128], [1, 3]]), 2.0)

        # ---- main loop: tiles of 128 batches (8 chunks of 16)
        gri_all = 0
        for t in range(nt):
            half = t % 2
            # transposes into the tta ring half
            for cc in range(8):
                c = 8 * t + cc
                slot = 256 * (8 * half + cc)
                tt = ttpool.tile([112, 256], F16)
                nc.tensor.transpose(
                    tt[:, 0:128],
                    cap(ctt1, 7 * CHUNK * c, [[1, 112]]),
                    ident16)
                nc.tensor.transpose(
                    tt[:, 128:256],
                    cap(ctt2, 7 * CHUNK * c, [[1, 112]]),
                    ident16)
                nc.scalar.copy(capp(tta, 0, 112, slot, [[1, 256]]), tt[:, :])
            # batched scatter: 4 DMAs, each moves 4 k-blocks of all 8 chunks
            xbd = xbd0 if half == 0 else xbd1
            tpitch = tta.tensor.shape[-1]
            xpitch = xbd.tensor.shape[-1]
            for k in range(CHUNK):
                src_ap = bass.AP(
                    tensor=tta.tensor,
                    offset=7 * k * tpitch + 256 * 8 * half + 128,
                    ap=[[tpitch, 7], [256, 8], [1, 128]])
                dst_ap = bass.AP(
                    tensor=xbd.tensor,
                    offset=7 * k * xpitch + 128 * k,
                    ap=[[xpitch, 7], [2048, 8], [1, 128]])
                eng = nc.sync if k % 2 == 0 else nc.scalar
                eng.dma_start(dst_ap, src_ap)
            # grams + min-reduce per chunk
            mm = mmp.tile([128, 16 * G * 8], F16)  # [128, 2048]
            for cc in range(8):
                slot = 256 * (8 * half + cc)
                for h in range(2):
                    gri = gri_all
                    gri_all += 1
                    gr = grpool.tile([128, 1024], F32)
                    for g2 in range(2):
                        nc.tensor.matmul(
                            gr[:, 512 * g2:512 * (g2 + 1)],
                            capp(tta, 0, 128, slot, [[1, 128]]),
                            capp(xbd, 0, 128,
                                 2048 * cc + 512 * (2 * h + g2), [[1, 512]]))
                    m2 = mm[:, 128 * (2 * cc + h):128 * (2 * cc + h) + 128]
                    if gri % 4 != 0:
                        # route B: ACT linear cast PSUM->fp16, then a DVE max
                        # tree over contiguous runs: gr col = b*128 + u*16 + g
                        # (level g at stride G), pairs (u, u+4),(u,u+2),(u,u+1)
                        yc = ycp.tile([128, 1024], F16)
                        nc.scalar.copy(yc[:, :], gr[:, :])
                        w1 = t1p.tile([128, 512], F16)
                        nc.vector.tensor_tensor(
                            tap(w1, 0, [[64, 8], [1, 64]]),
                            tap(yc, 0, [[128, 8], [1, 64]]),
                            tap(yc, 64, [[128, 8], [1, 64]]), ALU.max)
                        w2 = t2p.tile([128, 256], F16)
                        nc.vector.tensor_tensor(
                            tap(w2, 0, [[32, 8], [1, 32]]),
                            tap(w1, 0, [[64, 8], [1, 32]]),
                            tap(w1, 32, [[64, 8], [1, 32]]), ALU.max)
                        nc.vector.tensor_tensor(
                            tap(mm, 128 * (2 * cc + h), [[16, 8], [1, 16]]),
                            tap(w2, 0, [[32, 8], [1, 16]]),
                            tap(w2, 16, [[32, 8], [1, 16]]), ALU.max)
                    else:
                        # route A: DVE segmented max straight out of PSUM;
                        # level members at stride G (innermost dim u)
                        nc.vector.tensor_reduce(
                            m2.rearrange("p (b g) -> p b g", g=G),
                            tap(gr, 0, [[128, 8], [1, G], [G, SEG]]),
                            axis=AX.X, op=ALU.max)
            # stage2: sqrt, sub rbar, segmented max over levels
            s2 = s2p.tile([128, 2048], F16)
            nc.scalar.activation(s2[:, :], mm[:, :], AF.Sqrt,
                                 bias=epsb[0:128, 0:1], scale=-1.0)
            y2 = y2p.tile([128, 2048], F16)
            nc.gpsimd.tensor_tensor(
                y2[:, :].rearrange("p (b g) -> p b g", g=G),
                cap(rbar, 0, [[0, 128], [1, G]]),
                s2[:, :].rearrange("p (b g) -> p b g", g=G), ALU.subtract)
            z = zpool.tile([128, 128], F16)
            nc.vector.tensor_reduce(
                z[:, :], y2[:, :].rearrange("p (b g) -> p b g", g=G),
                axis=AX.X, op=ALU.max)
            # tail for this tile
            z2 = z2pool.tile([128, 128], F16)
            nc.vector.tensor_scalar_add(z2[:, :], z[:, :],
                                        rcolp[0:128, 0:1])
            ztr = prepool.tile([128, 128], F16, tag="pre")
            nc.tensor.transpose(ztr[:, :], z2[:, :], p16[0:128, 0:128])
            osb = outp.tile([128, L], F32)
            nc.vector.tensor_reduce(
                osb[:, :], ztr[:, :].rearrange("p (a b) -> p a b", a=L),
                axis=AX.X, op=ALU.max)
            nc.sync.dma_start(out_d[128 * t:128 * (t + 1), :], osb[:, :])

    nc.compile()
    return nc


def get_nc(nt):
    key = ("nc", nt)
    if key not in _CACHE:
        _CACHE[key] = _build_nc(nt)
    return _CACHE[key]


# ---------------------------------------------------------------- entry point
def kernel(q, fixed_rot, fixed_trans, joint_axes, link_spheres,
           collision_mask):
    from concourse.bass_utils import run_bass_kernel_spmd

    q = np.asarray(q, np.float32)
    bc = B // NCORES
    nt = bc // 128
    consts = _host_consts(fixed_rot, fixed_trans, joint_axes, link_spheres,
                          collision_mask, bc)
    nc = get_nc(nt)
    in_maps = []
    for c in range(NCORES):
        m = {"q": np.ascontiguousarray(q[c * bc:(c + 1) * bc]), **consts}
        in_maps.append(m)
    res = run_bass_kernel_spmd(nc, in_maps, list(range(NCORES)))
    out = np.concatenate([res.results[c]["out"] for c in range(NCORES)],
                         axis=0)
    return out.astype(np.float32)


# revision 15
# speedup vs baseline: 1.9683x; 1.2332x over previous
"""Trainium2 Bass kernel for nn_DifferentiableRobotModel (self-collision link
distances from batched forward kinematics).

Pure data parallel over the batch (rollout) dim: 8192 rollouts -> 1024/core
on 8 NeuronCores. All FK params / sphere tables / masks are tiny and
replicated.

v3 pipeline (per core, bc = 1024 batches, 128 spheres = 16 links x 8),
"quantized min-first" formulation:
  * Spheres are globally relabeled in r-sorted order (slot p = sphere
    pi[p]); 16 r-levels of 8 spheres each get a representative rbar_g.
    max_j (r_j - d_ij) ~= max_g (rbar_g - sqrt(min_{j in g} d2_ij)), so the
    expensive per-pair sqrt / subtract / mask passes collapse into one
    segmented-min over raw d2 straight out of PSUM.
  * The pair mask (|link_i - link_j| <= 1 band) is folded INTO the gram
    matmul: lhsT rows 112..127 hold BIG*[link(i)==m], rhs rows 112..127
    hold the band indicator [|m - link(j)| <= 1]; masked pairs get d2+BIG
    and never win the min. Zero extra PE cycles (matmul cost = columns).
  1. FK on DVE, batch-on-partitions (serial chain over 16 links using
     M_l(q) = P_l + sin(q) Q_l + (1-cos q) S_l, host-precomputed P/Q/S),
     recentered by t_link8.
  2. PE transposes S planes -> mall; world centers via bd2xp^T @ mall
     (bd2xp columns in slot order); derived ctt1/ctt2 attr planes with
     sqh/sql split of |c|^2 for fp32-grade d2 accuracy.
  3. Per 16-batch chunk: 2 PE transposes -> tta ring [128,256] (rows
     0..111 dynamic, 112..127 static mask); 4 batched scatter DMAs per
     128-batch tile build the block-diagonal X operand.
  4. Grams: 512-col fp16 matmuls -> d2' [128 i, 8 batch * 128 j] in PSUM.
  5. Per gram tile, two routes: (A) DVE segmented min (seg 8) PSUM->fp16,
     (B) ACT cast PSUM->fp16 + GpSimd 3-level pairwise-min tree.
     Both land m2[i, (b,g)] per tile.
  6. Stage2 per tile [128, 2048]: ACT sqrt(+eps), DVE sub rbar_g, DVE
     segmented max over g -> z[i,b].
  7. Tail: z + r_i, PE transpose with the inverse permutation matrix
     (un-relabels spheres for free), grouped max over each link's 8
     spheres -> out [batch, 16].
"""
import sys
import numpy as np

sys.path.insert(0, "/opt/trn_rl_repo")

import concourse.bass as bass  # noqa: E402
import concourse.tile as tile  # noqa: E402
from concourse import bacc, mybir  # noqa: E402
from contextlib import ExitStack  # noqa: E402

F32 = mybir.dt.float32
F16 = mybir.dt.float16
AF = mybir.ActivationFunctionType
ALU = mybir.AluOpType
AX = mybir.AxisListType

B, L, NS = 8192, 16, 8
N = L * NS              # 128 spheres
NCORES = 8
G = 16                  # r-quantization levels (8 spheres each, r-sorted)
SEG = N // G            # spheres per level
EPS = np.float32(4e-6)  # d2 positivity shift
BIG = np.float32(60.0)  # mask offset added to d2 of ignored pairs
CHUNK = 16              # batches per transpose/gram chunk (7*16=112 rows)
NB_ROUTE = 5            # of every 8 gram tiles, this many take the ACT route

_CACHE = {}


# ---------------------------------------------------------------- host consts
def _host_consts(fixed_rot, fixed_trans, joint_axes, link_spheres,
                 collision_mask, bc):
    f32, f16 = np.float32, np.float16
    ax = np.asarray(joint_axes, f32)
    K = np.zeros((L, 3, 3), f32)
    K[:, 0, 1], K[:, 0, 2] = -ax[:, 2], ax[:, 1]
    K[:, 1, 0], K[:, 1, 2] = ax[:, 2], -ax[:, 0]
    K[:, 2, 0], K[:, 2, 1] = -ax[:, 1], ax[:, 0]
    K2 = np.einsum("lij,ljk->lik", K, K).astype(f32)
    A = np.asarray(fixed_rot, f32)
    P = A
    Q = np.einsum("lij,ljk->lik", A, K).astype(f32)
    S = np.einsum("lij,ljk->lik", A, K2).astype(f32)

    # pqs [128, 576]: sections P/Q/S as 3x4 blocks, col sec*192 + l*12 +
    # j*4 + b4; b4==3 holds ftrans (P section) / 0 (Q,S sections)
    ft = np.asarray(fixed_trans, f32)
    pqs = np.zeros((128, 576), f32)
    for sec, Mx in enumerate((P, Q, S)):
        blk = np.zeros((L, 3, 4), f32)
        blk[:, :, :3] = Mx
        if sec == 0:
            blk[:, :, 3] = ft
        pqs[:, sec * 192:(sec + 1) * 192] = blk.reshape(1, L * 12)

    x = np.asarray(link_spheres, f32)[..., :3]           # [L,NS,3]
    r = np.asarray(link_spheres, f32)[..., 3].reshape(N)

    # slot relabeling: slot p holds original sphere pi[p]. Level g of slot
    # p is p % G (u-major interleave), so level members sit at stride G in
    # j and the min-tree reduces with contiguous-run access patterns.
    pr = np.argsort(r, kind="stable")                    # rank -> sphere
    pi = np.empty(N, np.int64)
    for p in range(N):
        pi[p] = pr[(p % G) * SEG + p // G]
    rs = r[pi]                                           # r per slot
    lnk = pi // NS                                       # link per slot
    rr = r[pr]
    rbar_g = np.empty(G, f32)
    for g in range(G):
        seg = rr[SEG * g:SEG * (g + 1)]
        rbar_g[g] = 0.5 * (seg.min() + seg.max())

    # bd2xp [64, 128] fp16: rows (l, m): slot cols; col p active on rows of
    # link lnk[p]
    bd2xp = np.zeros((64, N), f32)
    for p in range(N):
        j = pi[p]
        l = j // NS
        bd2xp[4 * l:4 * l + 3, p] = x[l, j % NS, :]
        bd2xp[4 * l + 3, p] = 1.0
    bd2xp = bd2xp.astype(f16)

    # mask tables (slot space). allowed = band AND collision_mask input.
    # The gram computes NEGATED distances (-d2); masked pairs get -BIG so
    # they lose every max. xbd mask rows hold -1, tta rows hold +BIG.
    cm = np.asarray(collision_mask)
    bandmask = np.zeros((L, N), f32)                     # [m, slot j]
    for m in range(L):
        for p in range(N):
            lj = lnk[p]
            ignored = (abs(m - lj) <= 1) or (not cm[m, lj])
            bandmask[m, p] = -1.0 if ignored else 0.0
    # xbdm [16, 16384]: band pattern tiled over 8 chunks * 16 batches
    xbdm = np.tile(bandmask, (1, 128)).astype(f16)
    # ttam [16, 4096]: 16 slots of 256; first 128 cols = BIG*[lnk==m]
    tslot = np.zeros((L, 256), f32)
    for m in range(L):
        tslot[m, :N] = BIG * (lnk == m)
    ttam = np.tile(tslot, (1, 16)).astype(f16)

    # rbar16 [128, G], rcolp [128, 1] (r per slot)
    rbar16 = np.tile(rbar_g.reshape(1, G), (128, 1)).astype(f16)
    rcolp = rs.reshape(N, 1).astype(f32)
    # tail un-permutation: ztr = z2^T @ p16; p16[p, c] = [pi[p] == c]
    p16 = np.zeros((N, N), f16)
    p16[np.arange(N), pi] = 1.0

    ident = np.eye(128, dtype=f32)
    ident16 = np.eye(128, dtype=f16)
    epsb = np.full((128, 1), EPS, f32)
    xzero = np.zeros((112, 16384), f16)
    return dict(pqs=pqs, bd2xp=bd2xp, xbdm=xbdm, ttam=ttam,
                rbar16=rbar16, rcolp=rcolp, p16=p16,
                ident=ident, ident16=ident16, epsb=epsb, xzero=xzero)


# ---------------------------------------------------------------- device build
def _build_nc(nt):
    """Build + compile the per-core Bass module for nt tiles of 128 batches."""
    bc = nt * 128
    nc = bacc.Bacc("TRN2", target_bir_lowering=False, debug=False,
                   num_devices=NCORES)

    q_d = nc.dram_tensor("q", [bc, L], F32, kind="ExternalInput").ap()
    pqs_d = nc.dram_tensor("pqs", [128, 576], F32, kind="ExternalInput").ap()
    bd2xp_d = nc.dram_tensor("bd2xp", [64, N], F16, kind="ExternalInput").ap()
    xbdm_d = nc.dram_tensor("xbdm", [16, 16384], F16,
                            kind="ExternalInput").ap()
    ttam_d = nc.dram_tensor("ttam", [16, 4096], F16, kind="ExternalInput").ap()
    rbar_d = nc.dram_tensor("rbar16", [128, G], F16, kind="ExternalInput").ap()
    rcolp_d = nc.dram_tensor("rcolp", [N, 1], F32, kind="ExternalInput").ap()
    p16_d = nc.dram_tensor("p16", [N, N], F16, kind="ExternalInput").ap()
    ident_d = nc.dram_tensor("ident", [128, 128], F32,
                             kind="ExternalInput").ap()
    ident16_d = nc.dram_tensor("ident16", [128, 128], F16,
                               kind="ExternalInput").ap()
    epsb_d = nc.dram_tensor("epsb", [128, 1], F32, kind="ExternalInput").ap()
    xzero_d = nc.dram_tensor("xzero", [112, 16384], F16,
                             kind="ExternalInput").ap()
    out_d = nc.dram_tensor("out", [bc, L], F32, kind="ExternalOutput").ap()

    # persistent SBUF tensors
    qsb = nc.alloc_sbuf_tensor("qsb", [128, 16 * nt], F32).ap()
    sinb = nc.alloc_sbuf_tensor("sinb", [128, 16 * nt], F32).ap()
    cosb = nc.alloc_sbuf_tensor("cosb", [128, 16 * nt], F32).ap()
    omcb = nc.alloc_sbuf_tensor("omcb", [128, 16 * nt], F32).ap()
    pqs = nc.alloc_sbuf_tensor("pqs_sb", [128, 576], F32).ap()
    bd2xp = nc.alloc_sbuf_tensor("bd2xp_sb", [64, N], F16).ap()
    rbar = nc.alloc_sbuf_tensor("rbar_sb", [128, G], F16).ap()
    rcolp = nc.alloc_sbuf_tensor("rcolp_sb", [N, 1], F32).ap()
    p16 = nc.alloc_sbuf_tensor("p16_sb", [128, 128], F16).ap()
    ident = nc.alloc_sbuf_tensor("ident_sb", [128, 128], F32).ap()
    ident16 = nc.alloc_sbuf_tensor("ident16_sb", [128, 128], F16).ap()
    epsb = nc.alloc_sbuf_tensor("epsb_sb", [128, 1], F32).ap()
    # FK state, homogeneous 3x4, plane-major: col = t*204 + a*68 + slot*4
    # + b4 holds H[a,b4] = [R | t]; slot 0 = identity pose. Plane-major so
    # the S-plane transpose read (slot, b4) collapses to one contiguous run.
    SP = 204
    sfk = nc.alloc_sbuf_tensor("sfk", [128, SP * nt], F32).ap()
    mw = nc.alloc_sbuf_tensor("mw", [128, 192 * nt], F32).ap()
    mw2 = nc.alloc_sbuf_tensor("mw2", [128, 192 * nt], F32).ap()
    tscr = nc.alloc_sbuf_tensor("tscr", [128, 12 * nt], F32).ap()
    # mall [64, 3*bc] fp16, batch-major per tile: col = b*3 + k
    mall = nc.alloc_sbuf_tensor("mall", [64, 3 * bc], F16).ap()
    # ctt1/ctt2 [128, 7*bc] fp16: col = b*7 + attr (contiguous per batch)
    # ctt1 (T1): 0-2 c, 3/4 one, 5 sqh, 6 sql
    # ctt2 (T2): 0-2 -2c, 3 sqh, 4 sql, 5/6 one
    ctt1 = nc.alloc_sbuf_tensor("ctt1", [128, 7 * bc], F16).ap()
    ctt2 = nc.alloc_sbuf_tensor("ctt2", [128, 7 * bc], F16).ap()
    # tta ring: 16 slots of [128, 256] (T1|T2 per chunk); rows 112..127
    # static mask rows (DMA'd once); halves alternate per tile
    tta = nc.alloc_sbuf_tensor("tta", [128, 16 * 256], F16).ap()
    # block-diag gram moving operands: per tile [128, 8*2048]
    # rows 0..111 zeros + scatter-DMA'd diagonal; rows 112..127 static band
    xbd0 = nc.alloc_sbuf_tensor("xbd0", [128, 16384], F16).ap()
    xbd1 = nc.alloc_sbuf_tensor("xbd1", [128, 16384], F16).ap()

    def cap(base, offset, dims):
        """Custom AP on a persistent tensor: dims = [[step,count],...] (free)."""
        pitch = base.tensor.shape[-1]
        nparts = base.tensor.shape[0]
        return bass.AP(tensor=base.tensor, offset=offset,
                       ap=[[pitch, nparts]] + list(dims))

    def capp(base, prow, nrow, offset, dims):
        """Custom AP with partition sub-range [prow, prow+nrow)."""
        pitch = base.tensor.shape[-1]
        return bass.AP(tensor=base.tensor, offset=prow * pitch + offset,
                       ap=[[pitch, nrow]] + list(dims))

    def tap(tl, off, dims):
        """Custom free-dim AP on a pool tile (keeps its partition dim)."""
        a = tl[:, :]
        return bass.AP(tensor=a.tensor, offset=a.offset + off,
                       ap=[list(a.ap[0])] + list(dims))

    with tile.TileContext(nc) as tc, ExitStack() as ctx:
        prepool = ctx.enter_context(tc.tile_pool(name="pre", bufs=1,
                                                 space="PSUM"))
        ttpool = ctx.enter_context(tc.tile_pool(name="ttp", bufs=2,
                                                space="PSUM"))
        grpool = ctx.enter_context(tc.tile_pool(name="gram", bufs=2,
                                                space="PSUM"))
        sqwp = ctx.enter_context(tc.tile_pool(name="sqw", bufs=2))
        ycp = ctx.enter_context(tc.tile_pool(name="yc", bufs=3))
        t1p = ctx.enter_context(tc.tile_pool(name="t1", bufs=2))
        t2p = ctx.enter_context(tc.tile_pool(name="t2", bufs=2))
        mmp = ctx.enter_context(tc.tile_pool(name="mm", bufs=2))
        s2p = ctx.enter_context(tc.tile_pool(name="s2", bufs=2))
        y2p = ctx.enter_context(tc.tile_pool(name="y2", bufs=2))
        zpool = ctx.enter_context(tc.tile_pool(name="z", bufs=2))
        z2pool = ctx.enter_context(tc.tile_pool(name="z2", bufs=2))
        outp = ctx.enter_context(tc.tile_pool(name="outsb", bufs=2))

        # ---- input DMAs
        nc.sync.dma_start(pqs, pqs_d)
        nc.sync.dma_start(bd2xp, bd2xp_d)
        nc.sync.dma_start(rbar, rbar_d)
        nc.sync.dma_start(rcolp, rcolp_d)
        nc.sync.dma_start(p16, p16_d)
        nc.sync.dma_start(ident, ident_d)
        nc.sync.dma_start(ident16, ident16_d)
        nc.sync.dma_start(epsb, epsb_d)
        # static mask rows 112..127 of tta / xbd0 / xbd1
        nc.sync.dma_start(capp(tta, 112, 16, 0, [[1, 4096]]), ttam_d)
        nc.sync.dma_start(capp(xbd0, 112, 16, 0, [[1, 16384]]), xbdm_d)
        nc.sync.dma_start(capp(xbd1, 112, 16, 0, [[1, 16384]]), xbdm_d)
        for t in range(nt):
            nc.sync.dma_start(cap(qsb, 16 * t, [[1, 16]]),
                              q_d[128 * t:128 * (t + 1), :])

        # ---- sin / cos / (1-cos)
        nc.scalar.activation(sinb, qsb, AF.Sin)
        # 1 - cos(q) = 2 sin^2(q/2); Sin LUT domain is [-pi, pi]
        nc.scalar.activation(cosb, qsb, AF.Sin, scale=0.5)
        nc.vector.tensor_mul(omcb, cosb, cosb)
        nc.vector.tensor_scalar_mul(omcb, omcb, 2.0)

        # ---- zero-fill: slot0 of sfk = identity pose [I | 0]
        nc.gpsimd.memset(cap(sfk, 0, [[SP, nt], [68, 3], [1, 4]]), 0.0)
        nc.gpsimd.memset(cap(sfk, 0, [[SP, nt], [69, 3]]), 1.0)  # I diag
        # const-1 planes
        nc.vector.memset(cap(ctt1, 3, [[7, bc], [1, 2]]), 1.0)
        nc.vector.memset(cap(ctt2, 5, [[7, bc], [1, 2]]), -1.0)
        # block-diag X dynamic rows zeros (written once; only diagonal blocks
        # rewritten by scatter DMAs)
        nc.sync.dma_start(capp(xbd0, 0, 112, 0, [[1, 16384]]), xzero_d)
        nc.sync.dma_start(capp(xbd1, 0, 112, 0, [[1, 16384]]), xzero_d)

        # ---- [M_l | f_l] = P4 + sin*Q4 + (1-cos)*S4: mw[(t,l,(j,b4))]
        # FK runs entirely on GpSimd; tile 0 is computed first so the main
        # per-tile pipeline can start while tiles 1..nt-1 FK completes.
        def fk_group(t0, tn):
            ntg = tn - t0
            mo, so, fo = 192 * t0, 16 * t0, SP * t0
            mdims = [[192, ntg], [12, L], [1, 12]]
            sdims = [[16, ntg], [1, L], [0, 12]]
            nc.gpsimd.tensor_tensor(cap(mw, mo, mdims),
                                    cap(pqs, 192, [[0, ntg]] + mdims[1:]),
                                    cap(sinb, so, sdims), ALU.mult)
            nc.gpsimd.tensor_tensor(cap(mw2, mo, mdims),
                                    cap(pqs, 384, [[0, ntg]] + mdims[1:]),
                                    cap(omcb, so, sdims), ALU.mult)
            nc.gpsimd.tensor_tensor(cap(mw, mo, mdims), cap(mw, mo, mdims),
                                    cap(mw2, mo, mdims), ALU.add)
            nc.gpsimd.tensor_tensor(cap(mw, mo, mdims), cap(mw, mo, mdims),
                                    cap(pqs, 0, [[0, ntg]] + mdims[1:]),
                                    ALU.add)
            for l in range(L):
                sp, s_ = 4 * l, 4 * (l + 1)   # prev slot, this slot (col/4)
                outH = cap(sfk, fo + s_, [[SP, ntg], [68, 3], [1, 4]])
                tmpH = cap(tscr, 12 * t0, [[12, ntg], [4, 3], [1, 4]])
                for j in range(3):
                    i0 = cap(sfk, fo + sp + j, [[SP, ntg], [68, 3], [0, 4]])
                    i1 = cap(mw, mo + 12 * l + 4 * j,
                             [[192, ntg], [0, 3], [1, 4]])
                    if j == 0:
                        nc.gpsimd.tensor_tensor(outH, i0, i1, ALU.mult)
                    else:
                        nc.gpsimd.tensor_tensor(tmpH, i0, i1, ALU.mult)
                        nc.gpsimd.tensor_tensor(outH, outH, tmpH, ALU.add)
                # t_l += t_p
                nc.gpsimd.tensor_tensor(
                    cap(sfk, fo + s_ + 3, [[SP, ntg], [68, 3]]),
                    cap(sfk, fo + s_ + 3, [[SP, ntg], [68, 3]]),
                    cap(sfk, fo + sp + 3, [[SP, ntg], [68, 3]]), ALU.add)
            # recenter: t'_l = t_l - t_link8 (slot 9)
            nc.gpsimd.tensor_copy(cap(tscr, 12 * t0, [[12, ntg], [1, 3]]),
                                  cap(sfk, fo + 4 * 9 + 3,
                                      [[SP, ntg], [68, 3]]))
            nc.gpsimd.tensor_tensor(
                cap(sfk, fo + 4 + 3, [[SP, ntg], [68, 3], [4, L]]),
                cap(sfk, fo + 4 + 3, [[SP, ntg], [68, 3], [4, L]]),
                cap(tscr, 12 * t0, [[12, ntg], [1, 3], [0, L]]),
                ALU.subtract)

        fk_group(0, 1)
        fk_group(1, nt)

        # ---- per tile: S planes k=0..2 -> mall; CT matmul -> ctt planes 0..2
        for t in range(nt):
            trm = prepool.tile([64, 384], F32, tag="pre")
            for k in range(3):
                nc.tensor.transpose(
                    trm[:, 128 * k:128 * (k + 1)],
                    cap(sfk, SP * t + 68 * k + 4, [[1, 64]]),
                    ident)
            # mall batch-major: col = b*3 + k  (trm col = k*128 + b)
            nc.scalar.copy(
                capp(mall, 0, 64, 3 * 128 * t, [[1, 3], [3, 128]]),
                trm[:, :])
            # world centers: ctp[j, (b,k)] = bd2xp^T @ mall
            ctp = prepool.tile([128, 384], F32, tag="pre")
            nc.tensor.matmul(
                ctp[:, :],
                bd2xp[0:64, :],
                capp(mall, 0, 64, 3 * 128 * t, [[1, 384]]))
            nc.scalar.copy(
                cap(ctt1, 7 * 128 * t, [[7, 128], [1, 3]]),
                ctp[:, :])
            # derived planes
            c_ap = cap(ctt1, 7 * 128 * t, [[7, 128], [1, 3]])
            sqw = sqwp.tile([128, 384], F32)
            nc.gpsimd.tensor_tensor(sqw[:, :], c_ap, c_ap, ALU.mult)
            sq32 = sqwp.tile([128, 128], F32)
            nc.vector.reduce_sum(
                sq32[:, :], sqw[:, :].rearrange("p (b k) -> p b k", k=3),
                axis=AX.X)
            # sqh (fp16) and sql = sq - sqh
            nc.gpsimd.tensor_copy(cap(ctt1, 7 * 128 * t + 5, [[7, 128]]),
                                  sq32[:, :])
            nc.vector.tensor_sub(cap(ctt1, 7 * 128 * t + 6, [[7, 128]]),
                                 sq32[:, :],
                                 cap(ctt1, 7 * 128 * t + 5, [[7, 128]]))
            nc.vector.tensor_scalar_mul(
                cap(ctt2, 7 * 128 * t + 3, [[7, 128], [1, 2]]),
                cap(ctt1, 7 * 128 * t + 5, [[7, 128], [1, 2]]), -1.0)
            nc.vector.tensor_scalar_mul(
                cap(ctt2, 7 * 128 * t + 0, [[7, 128], [1, 3]]),
                cap(ctt1, 7 * 128 * t + 0, [[7, 128], [1, 3]]), 2.0)

        # ---- main loop: tiles of 128 batches (8 chunks of 16)
        gri_all = 0
        for t in range(nt):
            half = t % 2
            # transposes into the tta ring half
            for cc in range(8):
                c = 8 * t + cc
                slot = 256 * (8 * half + cc)
                tt = ttpool.tile([112, 256], F16)
                nc.tensor.transpose(
                    tt[:, 0:128],
                    cap(ctt1, 7 * CHUNK * c, [[1, 112]]),
                    ident16)
                nc.tensor.transpose(
                    tt[:, 128:256],
                    cap(ctt2, 7 * CHUNK * c, [[1, 112]]),
                    ident16)
                nc.scalar.copy(capp(tta, 0, 112, slot, [[1, 256]]), tt[:, :])
            # batched scatter: 4 DMAs, each moves 4 k-blocks of all 8 chunks
            xbd = xbd0 if half == 0 else xbd1
            tpitch = tta.tensor.shape[-1]
            xpitch = xbd.tensor.shape[-1]
            for k in range(CHUNK):
                src_ap = bass.AP(
                    tensor=tta.tensor,
                    offset=7 * k * tpitch + 256 * 8 * half + 128,
                    ap=[[tpitch, 7], [256, 8], [1, 128]])
                dst_ap = bass.AP(
                    tensor=xbd.tensor,
                    offset=7 * k * xpitch + 128 * k,
                    ap=[[xpitch, 7], [2048, 8], [1, 128]])
                nc.sync.dma_start(dst_ap, src_ap)
            # grams + min-reduce per chunk
            mm = mmp.tile([128, 16 * G * 8], F16)  # [128, 2048]
            for cc in range(8):
                slot = 256 * (8 * half + cc)
                for h in range(2):
                    gri = gri_all
                    gri_all += 1
                    gr = grpool.tile([128, 1024], F32)
                    for g2 in range(2):
                        nc.tensor.matmul(
                            gr[:, 512 * g2:512 * (g2 + 1)],
                            capp(tta, 0, 128, slot, [[1, 128]]),
                            capp(xbd, 0, 128,
                                 2048 * cc + 512 * (2 * h + g2), [[1, 512]]))
                    m2 = mm[:, 128 * (2 * cc + h):128 * (2 * cc + h) + 128]
                    if gri % 5 >= 2:
                        # route B: ACT linear cast PSUM->fp16, then a DVE max
                        # tree over contiguous runs: gr col = b*128 + u*16 + g
                        # (level g at stride G), pairs (u, u+4),(u,u+2),(u,u+1)
                        yc = ycp.tile([128, 1024], F16)
                        nc.scalar.copy(yc[:, :], gr[:, :])
                        w1 = t1p.tile([128, 512], F16)
                        nc.vector.tensor_tensor(
                            tap(w1, 0, [[64, 8], [1, 64]]),
                            tap(yc, 0, [[128, 8], [1, 64]]),
                            tap(yc, 64, [[128, 8], [1, 64]]), ALU.max)
                        w2 = t2p.tile([128, 256], F16)
                        nc.vector.tensor_tensor(
                            tap(w2, 0, [[32, 8], [1, 32]]),
                            tap(w1, 0, [[64, 8], [1, 32]]),
                            tap(w1, 32, [[64, 8], [1, 32]]), ALU.max)
                        nc.vector.tensor_tensor(
                            tap(mm, 128 * (2 * cc + h), [[16, 8], [1, 16]]),
                            tap(w2, 0, [[32, 8], [1, 16]]),
                            tap(w2, 16, [[32, 8], [1, 16]]), ALU.max)
                    else:
                        # route A: DVE segmented max straight out of PSUM;
                        # level members at stride G (innermost dim u)
                        nc.vector.tensor_reduce(
                            m2.rearrange("p (b g) -> p b g", g=G),
                            tap(gr, 0, [[128, 8], [1, G], [G, SEG]]),
                            axis=AX.X, op=ALU.max)
            # stage2: sqrt, sub rbar, segmented max over levels
            s2 = s2p.tile([128, 2048], F16)
            nc.scalar.activation(s2[:, :], mm[:, :], AF.Sqrt,
                                 bias=epsb[0:128, 0:1], scale=-1.0)
            y2 = y2p.tile([128, 2048], F16)
            nc.gpsimd.tensor_tensor(
                y2[:, :].rearrange("p (b g) -> p b g", g=G),
                cap(rbar, 0, [[0, 128], [1, G]]),
                s2[:, :].rearrange("p (b g) -> p b g", g=G), ALU.subtract)
            z = zpool.tile([128, 128], F16)
            nc.vector.tensor_reduce(
                z[:, :], y2[:, :].rearrange("p (b g) -> p b g", g=G),
                axis=AX.X, op=ALU.max)
            # tail for this tile
            z2 = z2pool.tile([128, 128], F16)
            nc.vector.tensor_scalar_add(z2[:, :], z[:, :],
                                        rcolp[0:128, 0:1])
            ztr = prepool.tile([128, 128], F16, tag="pre")
            nc.tensor.transpose(ztr[:, :], z2[:, :], p16[0:128, 0:128])
            osb = outp.tile([128, L], F32)
            nc.vector.tensor_reduce(
                osb[:, :], ztr[:, :].rearrange("p (a b) -> p a b", a=L),
                axis=AX.X, op=ALU.max)
            nc.sync.dma_start(out_d[128 * t:128 * (t + 1), :], osb[:, :])

    nc.compile()
    return nc


def get_nc(nt):
    key = ("nc", nt)
    if key not in _CACHE:
        _CACHE[key] = _build_nc(nt)
    return _CACHE[key]


# ---------------------------------------------------------------- entry point
def kernel(q, fixed_rot, fixed_trans, joint_axes, link_spheres,
           collision_mask):
    from concourse.bass_utils import run_bass_kernel_spmd

    q = np.asarray(q, np.float32)
    bc = B // NCORES
    nt = bc // 128
    consts = _host_consts(fixed_rot, fixed_trans, joint_axes, link_spheres,
                          collision_mask, bc)
    nc = get_nc(nt)
    in_maps = []
    for c in range(NCORES):
        m = {"q": np.ascontiguousarray(q[c * bc:(c + 1) * bc]), **consts}
        in_maps.append(m)
    res = run_bass_kernel_spmd(nc, in_maps, list(range(NCORES)))
    out = np.concatenate([res.results[c]["out"] for c in range(NCORES)],
                         axis=0)
    return out.astype(np.float32)


# revision 16
# speedup vs baseline: 2.2166x; 1.1261x over previous
"""Trainium2 Bass kernel for nn_DifferentiableRobotModel (self-collision link
distances from batched forward kinematics).

Pure data parallel over the batch (rollout) dim: 8192 rollouts -> 1024/core
on 8 NeuronCores. All FK params / sphere tables / masks are tiny and
replicated.

v3 pipeline (per core, bc = 1024 batches, 128 spheres = 16 links x 8),
"quantized min-first" formulation:
  * Spheres are globally relabeled in r-sorted order (slot p = sphere
    pi[p]); 16 r-levels of 8 spheres each get a representative rbar_g.
    max_j (r_j - d_ij) ~= max_g (rbar_g - sqrt(min_{j in g} d2_ij)), so the
    expensive per-pair sqrt / subtract / mask passes collapse into one
    segmented-min over raw d2 straight out of PSUM.
  * The pair mask (|link_i - link_j| <= 1 band) is folded INTO the gram
    matmul: lhsT rows 112..127 hold BIG*[link(i)==m], rhs rows 112..127
    hold the band indicator [|m - link(j)| <= 1]; masked pairs get d2+BIG
    and never win the min. Zero extra PE cycles (matmul cost = columns).
  1. FK on DVE, batch-on-partitions (serial chain over 16 links using
     M_l(q) = P_l + sin(q) Q_l + (1-cos q) S_l, host-precomputed P/Q/S),
     recentered by t_link8.
  2. PE transposes S planes -> mall; world centers via bd2xp^T @ mall
     (bd2xp columns in slot order); derived ctt1/ctt2 attr planes with
     sqh/sql split of |c|^2 for fp32-grade d2 accuracy.
  3. Per 16-batch chunk: 2 PE transposes -> tta ring [128,256] (rows
     0..111 dynamic, 112..127 static mask); 4 batched scatter DMAs per
     128-batch tile build the block-diagonal X operand.
  4. Grams: 512-col fp16 matmuls -> d2' [128 i, 8 batch * 128 j] in PSUM.
  5. Per gram tile, two routes: (A) DVE segmented min (seg 8) PSUM->fp16,
     (B) ACT cast PSUM->fp16 + GpSimd 3-level pairwise-min tree.
     Both land m2[i, (b,g)] per tile.
  6. Stage2 per tile [128, 2048]: ACT sqrt(+eps), DVE sub rbar_g, DVE
     segmented max over g -> z[i,b].
  7. Tail: z + r_i, PE transpose with the inverse permutation matrix
     (un-relabels spheres for free), grouped max over each link's 8
     spheres -> out [batch, 16].
"""
import sys
import numpy as np

sys.path.insert(0, "/opt/trn_rl_repo")

import concourse.bass as bass  # noqa: E402
import concourse.tile as tile  # noqa: E402
from concourse import bacc, mybir  # noqa: E402
from contextlib import ExitStack  # noqa: E402

F32 = mybir.dt.float32
F16 = mybir.dt.float16
AF = mybir.ActivationFunctionType
ALU = mybir.AluOpType
AX = mybir.AxisListType

B, L, NS = 8192, 16, 8
N = L * NS              # 128 spheres
NCORES = 8
G = 16                  # r-quantization levels (8 spheres each, r-sorted)
SEG = N // G            # spheres per level
EPS = np.float32(4e-6)  # d2 positivity shift
BIG = np.float32(60.0)  # mask offset added to d2 of ignored pairs
CHUNK = 16              # batches per transpose/gram chunk (7*16=112 rows)
NB_ROUTE = 5            # of every 8 gram tiles, this many take the ACT route

_CACHE = {}


# ---------------------------------------------------------------- host consts
def _host_consts(fixed_rot, fixed_trans, joint_axes, link_spheres,
                 collision_mask, bc):
    f32, f16 = np.float32, np.float16
    ax = np.asarray(joint_axes, f32)
    K = np.zeros((L, 3, 3), f32)
    K[:, 0, 1], K[:, 0, 2] = -ax[:, 2], ax[:, 1]
    K[:, 1, 0], K[:, 1, 2] = ax[:, 2], -ax[:, 0]
    K[:, 2, 0], K[:, 2, 1] = -ax[:, 1], ax[:, 0]
    K2 = np.einsum("lij,ljk->lik", K, K).astype(f32)
    A = np.asarray(fixed_rot, f32)
    P = A
    Q = np.einsum("lij,ljk->lik", A, K).astype(f32)
    S = np.einsum("lij,ljk->lik", A, K2).astype(f32)

    # pqs [128, 576]: sections P/Q/S as 3x4 blocks, col sec*192 + l*12 +
    # j*4 + b4; b4==3 holds ftrans (P section) / 0 (Q,S sections)
    ft = np.asarray(fixed_trans, f32)
    pqs = np.zeros((128, 576), f32)
    for sec, Mx in enumerate((P, Q, S)):
        blk = np.zeros((L, 3, 4), f32)
        blk[:, :, :3] = Mx
        if sec == 0:
            blk[:, :, 3] = ft
        pqs[:, sec * 192:(sec + 1) * 192] = blk.reshape(1, L * 12)

    x = np.asarray(link_spheres, f32)[..., :3]           # [L,NS,3]
    r = np.asarray(link_spheres, f32)[..., 3].reshape(N)

    # slot relabeling: slot p holds original sphere pi[p]. Level g of slot
    # p is p % G (u-major interleave), so level members sit at stride G in
    # j and the min-tree reduces with contiguous-run access patterns.
    pr = np.argsort(r, kind="stable")                    # rank -> sphere
    pi = np.empty(N, np.int64)
    for p in range(N):
        pi[p] = pr[(p % G) * SEG + p // G]
    rs = r[pi]                                           # r per slot
    lnk = pi // NS                                       # link per slot
    rr = r[pr]
    rbar_g = np.empty(G, f32)
    for g in range(G):
        seg = rr[SEG * g:SEG * (g + 1)]
        rbar_g[g] = 0.5 * (seg.min() + seg.max())

    # bd2xp [64, 128] fp16: rows (l, m): slot cols; col p active on rows of
    # link lnk[p]
    bd2xp = np.zeros((64, N), f32)
    for p in range(N):
        j = pi[p]
        l = j // NS
        bd2xp[4 * l:4 * l + 3, p] = x[l, j % NS, :]
        bd2xp[4 * l + 3, p] = 1.0
    bd2xp = bd2xp.astype(f16)

    # mask tables (slot space). allowed = band AND collision_mask input.
    # The gram computes NEGATED distances (-d2); masked pairs get -BIG so
    # they lose every max. xbd mask rows hold -1, tta rows hold +BIG.
    cm = np.asarray(collision_mask)
    bandmask = np.zeros((L, N), f32)                     # [m, slot j]
    for m in range(L):
        for p in range(N):
            lj = lnk[p]
            ignored = (abs(m - lj) <= 1) or (not cm[m, lj])
            bandmask[m, p] = -1.0 if ignored else 0.0
    # xbdm [16, 16384]: band pattern tiled over 8 chunks * 16 batches
    xbdm = np.tile(bandmask, (1, 128)).astype(f16)
    # ttam [16, 4096]: 16 slots of 256; first 128 cols = BIG*[lnk==m]
    tslot = np.zeros((L, 256), f32)
    for m in range(L):
        tslot[m, :N] = BIG * (lnk == m)
    ttam = np.tile(tslot, (1, 16)).astype(f16)

    # rbar16 [128, G], rcolp [128, 1] (r per slot)
    rbar16 = np.tile(rbar_g.reshape(1, G), (128, 1)).astype(f16)
    rcolp = rs.reshape(N, 1).astype(f32)
    # tail un-permutation: ztr = z2^T @ p16; p16[p, c] = [pi[p] == c]
    p16 = np.zeros((N, N), f16)
    p16[np.arange(N), pi] = 1.0

    ident = np.eye(128, dtype=f32)
    ident16 = np.eye(128, dtype=f16)
    epsb = np.full((128, 1), EPS, f32)
    xzero = np.zeros((112, 16384), f16)
    return dict(pqs=pqs, bd2xp=bd2xp, xbdm=xbdm, ttam=ttam,
                rbar16=rbar16, rcolp=rcolp, p16=p16,
                ident=ident, ident16=ident16, epsb=epsb, xzero=xzero)


# ---------------------------------------------------------------- device build
def _build_nc(nt):
    """Build + compile the per-core Bass module for nt tiles of 128 batches."""
    bc = nt * 128
    nc = bacc.Bacc("TRN2", target_bir_lowering=False, debug=False,
                   num_devices=NCORES)

    q_d = nc.dram_tensor("q", [bc, L], F32, kind="ExternalInput").ap()
    pqs_d = nc.dram_tensor("pqs", [128, 576], F32, kind="ExternalInput").ap()
    bd2xp_d = nc.dram_tensor("bd2xp", [64, N], F16, kind="ExternalInput").ap()
    xbdm_d = nc.dram_tensor("xbdm", [16, 16384], F16,
                            kind="ExternalInput").ap()
    ttam_d = nc.dram_tensor("ttam", [16, 4096], F16, kind="ExternalInput").ap()
    rbar_d = nc.dram_tensor("rbar16", [128, G], F16, kind="ExternalInput").ap()
    rcolp_d = nc.dram_tensor("rcolp", [N, 1], F32, kind="ExternalInput").ap()
    p16_d = nc.dram_tensor("p16", [N, N], F16, kind="ExternalInput").ap()
    ident_d = nc.dram_tensor("ident", [128, 128], F32,
                             kind="ExternalInput").ap()
    ident16_d = nc.dram_tensor("ident16", [128, 128], F16,
                               kind="ExternalInput").ap()
    epsb_d = nc.dram_tensor("epsb", [128, 1], F32, kind="ExternalInput").ap()
    xzero_d = nc.dram_tensor("xzero", [112, 16384], F16,
                             kind="ExternalInput").ap()
    out_d = nc.dram_tensor("out", [bc, L], F32, kind="ExternalOutput").ap()

    # persistent SBUF tensors
    qsb = nc.alloc_sbuf_tensor("qsb", [128, 16 * nt], F32).ap()
    sinb = nc.alloc_sbuf_tensor("sinb", [128, 16 * nt], F32).ap()
    cosb = nc.alloc_sbuf_tensor("cosb", [128, 16 * nt], F32).ap()
    omcb = nc.alloc_sbuf_tensor("omcb", [128, 16 * nt], F32).ap()
    pqs = nc.alloc_sbuf_tensor("pqs_sb", [128, 576], F32).ap()
    bd2xp = nc.alloc_sbuf_tensor("bd2xp_sb", [64, N], F16).ap()
    rbar = nc.alloc_sbuf_tensor("rbar_sb", [128, G], F16).ap()
    rcolp = nc.alloc_sbuf_tensor("rcolp_sb", [N, 1], F32).ap()
    p16 = nc.alloc_sbuf_tensor("p16_sb", [128, 128], F16).ap()
    ident = nc.alloc_sbuf_tensor("ident_sb", [128, 128], F32).ap()
    ident16 = nc.alloc_sbuf_tensor("ident16_sb", [128, 128], F16).ap()
    epsb = nc.alloc_sbuf_tensor("epsb_sb", [128, 1], F32).ap()
    # FK state, homogeneous 3x4, plane-major: col = t*204 + a*68 + slot*4
    # + b4 holds H[a,b4] = [R | t]; slot 0 = identity pose. Plane-major so
    # the S-plane transpose read (slot, b4) collapses to one contiguous run.
    SP = 204
    sfk = nc.alloc_sbuf_tensor("sfk", [128, SP * nt], F32).ap()
    mw = nc.alloc_sbuf_tensor("mw", [128, 192 * nt], F32).ap()
    mw2 = nc.alloc_sbuf_tensor("mw2", [128, 192 * nt], F32).ap()
    tscr = nc.alloc_sbuf_tensor("tscr", [128, 12 * nt], F32).ap()
    # mall [64, 3*bc] fp16, batch-major per tile: col = b*3 + k
    mall = nc.alloc_sbuf_tensor("mall", [64, 3 * bc], F16).ap()
    # ctt1/ctt2 [128, 7*bc] fp16: col = b*7 + attr (contiguous per batch)
    # ctt1 (T1): 0-2 c, 3/4 one, 5 sqh, 6 sql
    # ctt2 (T2): 0-2 -2c, 3 sqh, 4 sql, 5/6 one
    ctt1 = nc.alloc_sbuf_tensor("ctt1", [128, 7 * bc], F16).ap()
    ctt2 = nc.alloc_sbuf_tensor("ctt2", [128, 7 * bc], F16).ap()
    # tta ring: 16 slots of [128, 256] (T1|T2 per chunk); rows 112..127
    # static mask rows (DMA'd once); halves alternate per tile
    tta = nc.alloc_sbuf_tensor("tta", [128, 16 * 256], F16).ap()
    # block-diag gram moving operands: per tile [128, 8*2048]
    # rows 0..111 zeros + scatter-DMA'd diagonal; rows 112..127 static band
    xbd0 = nc.alloc_sbuf_tensor("xbd0", [128, 16384], F16).ap()
    xbd1 = nc.alloc_sbuf_tensor("xbd1", [128, 16384], F16).ap()

    def cap(base, offset, dims):
        """Custom AP on a persistent tensor: dims = [[step,count],...] (free)."""
        pitch = base.tensor.shape[-1]
        nparts = base.tensor.shape[0]
        return bass.AP(tensor=base.tensor, offset=offset,
                       ap=[[pitch, nparts]] + list(dims))

    def capp(base, prow, nrow, offset, dims):
        """Custom AP with partition sub-range [prow, prow+nrow)."""
        pitch = base.tensor.shape[-1]
        return bass.AP(tensor=base.tensor, offset=prow * pitch + offset,
                       ap=[[pitch, nrow]] + list(dims))

    def tap(tl, off, dims):
        """Custom free-dim AP on a pool tile (keeps its partition dim)."""
        a = tl[:, :]
        return bass.AP(tensor=a.tensor, offset=a.offset + off,
                       ap=[list(a.ap[0])] + list(dims))

    with tile.TileContext(nc) as tc, ExitStack() as ctx:
        prepool = ctx.enter_context(tc.tile_pool(name="pre", bufs=1,
                                                 space="PSUM"))
        ttpool = ctx.enter_context(tc.tile_pool(name="ttp", bufs=2,
                                                space="PSUM"))
        grpool = ctx.enter_context(tc.tile_pool(name="gram", bufs=2,
                                                space="PSUM"))
        sqwp = ctx.enter_context(tc.tile_pool(name="sqw", bufs=2))
        ycp = ctx.enter_context(tc.tile_pool(name="yc", bufs=3))
        t1p = ctx.enter_context(tc.tile_pool(name="t1", bufs=2))
        t2p = ctx.enter_context(tc.tile_pool(name="t2", bufs=2))
        mmp = ctx.enter_context(tc.tile_pool(name="mm", bufs=2))
        s2p = ctx.enter_context(tc.tile_pool(name="s2", bufs=2))
        y2p = ctx.enter_context(tc.tile_pool(name="y2", bufs=2))
        zpool = ctx.enter_context(tc.tile_pool(name="z", bufs=2))
        z2pool = ctx.enter_context(tc.tile_pool(name="z2", bufs=2))
        outp = ctx.enter_context(tc.tile_pool(name="outsb", bufs=2))

        # ---- input DMAs
        nc.sync.dma_start(pqs, pqs_d)
        nc.sync.dma_start(bd2xp, bd2xp_d)
        nc.sync.dma_start(rbar, rbar_d)
        nc.sync.dma_start(rcolp, rcolp_d)
        nc.sync.dma_start(p16, p16_d)
        nc.sync.dma_start(ident, ident_d)
        nc.sync.dma_start(ident16, ident16_d)
        nc.sync.dma_start(epsb, epsb_d)
        # static mask rows 112..127 of tta / xbd0 / xbd1
        nc.sync.dma_start(capp(tta, 112, 16, 0, [[1, 4096]]), ttam_d)
        nc.sync.dma_start(capp(xbd0, 112, 16, 0, [[1, 16384]]), xbdm_d)
        nc.sync.dma_start(capp(xbd1, 112, 16, 0, [[1, 16384]]), xbdm_d)
        for t in range(nt):
            nc.sync.dma_start(cap(qsb, 16 * t, [[1, 16]]),
                              q_d[128 * t:128 * (t + 1), :])

        # ---- sin / cos / (1-cos)
        nc.scalar.activation(sinb, qsb, AF.Sin)
        # 1 - cos(q) = 2 sin^2(q/2); Sin LUT domain is [-pi, pi]
        nc.scalar.activation(cosb, qsb, AF.Sin, scale=0.5)
        nc.vector.tensor_mul(omcb, cosb, cosb)
        nc.vector.tensor_scalar_mul(omcb, omcb, 2.0)

        # ---- zero-fill: slot0 of sfk = identity pose [I | 0]
        nc.gpsimd.memset(cap(sfk, 0, [[SP, nt], [68, 3], [1, 4]]), 0.0)
        nc.gpsimd.memset(cap(sfk, 0, [[SP, nt], [69, 3]]), 1.0)  # I diag
        # const-1 planes
        nc.vector.memset(cap(ctt1, 3, [[7, bc], [1, 2]]), 1.0)
        nc.vector.memset(cap(ctt2, 5, [[7, bc], [1, 2]]), -1.0)
        # block-diag X dynamic rows zeros (written once; only diagonal blocks
        # rewritten by scatter DMAs)
        nc.sync.dma_start(capp(xbd0, 0, 112, 0, [[1, 16384]]), xzero_d)
        nc.sync.dma_start(capp(xbd1, 0, 112, 0, [[1, 16384]]), xzero_d)

        # ---- [M_l | f_l] = P4 + sin*Q4 + (1-cos)*S4: mw[(t,l,(j,b4))]
        # FK runs entirely on GpSimd; tile 0 is computed first so the main
        # per-tile pipeline can start while tiles 1..nt-1 FK completes.
        def fk_group(t0, tn, eng):
            ntg = tn - t0
            mo, so, fo = 192 * t0, 16 * t0, SP * t0
            mdims = [[192, ntg], [12, L], [1, 12]]
            sdims = [[16, ntg], [1, L], [0, 12]]
            eng.tensor_tensor(cap(mw, mo, mdims),
                              cap(pqs, 192, [[0, ntg]] + mdims[1:]),
                              cap(sinb, so, sdims), ALU.mult)
            eng.tensor_tensor(cap(mw2, mo, mdims),
                              cap(pqs, 384, [[0, ntg]] + mdims[1:]),
                              cap(omcb, so, sdims), ALU.mult)
            eng.tensor_tensor(cap(mw, mo, mdims), cap(mw, mo, mdims),
                              cap(mw2, mo, mdims), ALU.add)
            eng.tensor_tensor(cap(mw, mo, mdims), cap(mw, mo, mdims),
                              cap(pqs, 0, [[0, ntg]] + mdims[1:]),
                              ALU.add)
            for l in range(L):
                sp, s_ = 4 * l, 4 * (l + 1)   # prev slot, this slot (col/4)
                outH = cap(sfk, fo + s_, [[SP, ntg], [68, 3], [1, 4]])
                tmpH = cap(tscr, 12 * t0, [[12, ntg], [4, 3], [1, 4]])
                for j in range(3):
                    i0 = cap(sfk, fo + sp + j, [[SP, ntg], [68, 3], [0, 4]])
                    i1 = cap(mw, mo + 12 * l + 4 * j,
                             [[192, ntg], [0, 3], [1, 4]])
                    if j == 0:
                        eng.tensor_tensor(outH, i0, i1, ALU.mult)
                    else:
                        eng.tensor_tensor(tmpH, i0, i1, ALU.mult)
                        eng.tensor_tensor(outH, outH, tmpH, ALU.add)
                # t_l += t_p
                eng.tensor_tensor(
                    cap(sfk, fo + s_ + 3, [[SP, ntg], [68, 3]]),
                    cap(sfk, fo + s_ + 3, [[SP, ntg], [68, 3]]),
                    cap(sfk, fo + sp + 3, [[SP, ntg], [68, 3]]), ALU.add)
            # recenter: t'_l = t_l - t_link8 (slot 9)
            eng.tensor_copy(cap(tscr, 12 * t0, [[12, ntg], [1, 3]]),
                            cap(sfk, fo + 4 * 9 + 3,
                                [[SP, ntg], [68, 3]]))
            eng.tensor_tensor(
                cap(sfk, fo + 4 + 3, [[SP, ntg], [68, 3], [4, L]]),
                cap(sfk, fo + 4 + 3, [[SP, ntg], [68, 3], [4, L]]),
                cap(tscr, 12 * t0, [[12, ntg], [1, 3], [0, L]]),
                ALU.subtract)

        fk_group(0, 1, nc.vector)
        fk_group(1, nt, nc.gpsimd)

        # ---- per tile: S planes k=0..2 -> mall; CT matmul -> ctt planes 0..2
        for t in range(nt):
            trm = prepool.tile([64, 384], F32, tag="pre")
            for k in range(3):
                nc.tensor.transpose(
                    trm[:, 128 * k:128 * (k + 1)],
                    cap(sfk, SP * t + 68 * k + 4, [[1, 64]]),
                    ident)
            # mall batch-major: col = b*3 + k  (trm col = k*128 + b)
            nc.scalar.copy(
                capp(mall, 0, 64, 3 * 128 * t, [[1, 3], [3, 128]]),
                trm[:, :])
            # world centers: ctp[j, (b,k)] = bd2xp^T @ mall
            ctp = prepool.tile([128, 384], F32, tag="pre")
            nc.tensor.matmul(
                ctp[:, :],
                bd2xp[0:64, :],
                capp(mall, 0, 64, 3 * 128 * t, [[1, 384]]))
            nc.scalar.copy(
                cap(ctt1, 7 * 128 * t, [[7, 128], [1, 3]]),
                ctp[:, :])
            # derived planes
            c_ap = cap(ctt1, 7 * 128 * t, [[7, 128], [1, 3]])
            sqw = sqwp.tile([128, 384], F32)
            nc.gpsimd.tensor_tensor(sqw[:, :], c_ap, c_ap, ALU.mult)
            sq32 = sqwp.tile([128, 128], F32)
            nc.vector.reduce_sum(
                sq32[:, :], sqw[:, :].rearrange("p (b k) -> p b k", k=3),
                axis=AX.X)
            # sqh (fp16) and sql = sq - sqh
            nc.gpsimd.tensor_copy(cap(ctt1, 7 * 128 * t + 5, [[7, 128]]),
                                  sq32[:, :])
            nc.vector.tensor_sub(cap(ctt1, 7 * 128 * t + 6, [[7, 128]]),
                                 sq32[:, :],
                                 cap(ctt1, 7 * 128 * t + 5, [[7, 128]]))
            nc.vector.tensor_scalar_mul(
                cap(ctt2, 7 * 128 * t + 3, [[7, 128], [1, 2]]),
                cap(ctt1, 7 * 128 * t + 5, [[7, 128], [1, 2]]), -1.0)
            nc.vector.tensor_scalar_mul(
                cap(ctt2, 7 * 128 * t + 0, [[7, 128], [1, 3]]),
                cap(ctt1, 7 * 128 * t + 0, [[7, 128], [1, 3]]), 2.0)

        # ---- main loop: tiles of 128 batches (8 chunks of 16)
        gri_all = 0
        for t in range(nt):
            half = t % 2
            # transposes into the tta ring half
            for cc in range(8):
                c = 8 * t + cc
                slot = 256 * (8 * half + cc)
                tt = ttpool.tile([112, 256], F16)
                nc.tensor.transpose(
                    tt[:, 0:128],
                    cap(ctt1, 7 * CHUNK * c, [[1, 112]]),
                    ident16)
                nc.tensor.transpose(
                    tt[:, 128:256],
                    cap(ctt2, 7 * CHUNK * c, [[1, 112]]),
                    ident16)
                nc.scalar.copy(capp(tta, 0, 112, slot, [[1, 256]]), tt[:, :])
            # batched scatter: 4 DMAs, each moves 4 k-blocks of all 8 chunks
            xbd = xbd0 if half == 0 else xbd1
            tpitch = tta.tensor.shape[-1]
            xpitch = xbd.tensor.shape[-1]
            for k in range(CHUNK):
                src_ap = bass.AP(
                    tensor=tta.tensor,
                    offset=7 * k * tpitch + 256 * 8 * half + 128,
                    ap=[[tpitch, 7], [256, 8], [1, 128]])
                dst_ap = bass.AP(
                    tensor=xbd.tensor,
                    offset=7 * k * xpitch + 128 * k,
                    ap=[[xpitch, 7], [2048, 8], [1, 128]])
                eng = nc.sync if k % 2 == 0 else nc.gpsimd
                eng.dma_start(dst_ap, src_ap)
            # grams + min-reduce per chunk
            mm = mmp.tile([128, 16 * G * 8], F16)  # [128, 2048]
            for cc in range(8):
                slot = 256 * (8 * half + cc)
                for h in range(2):
                    gri = gri_all
                    gri_all += 1
                    gr = grpool.tile([128, 1024], F32)
                    for g2 in range(2):
                        nc.tensor.matmul(
                            gr[:, 512 * g2:512 * (g2 + 1)],
                            capp(tta, 0, 128, slot, [[1, 128]]),
                            capp(xbd, 0, 128,
                                 2048 * cc + 512 * (2 * h + g2), [[1, 512]]))
                    m2 = mm[:, 128 * (2 * cc + h):128 * (2 * cc + h) + 128]
                    if gri % 5 >= 2:
                        # route B: ACT linear cast PSUM->fp16, then a DVE max
                        # tree over contiguous runs: gr col = b*128 + u*16 + g
                        # (level g at stride G), pairs (u, u+4),(u,u+2),(u,u+1)
                        yc = ycp.tile([128, 1024], F16)
                        nc.scalar.copy(yc[:, :], gr[:, :])
                        w1 = t1p.tile([128, 512], F16)
                        nc.vector.tensor_tensor(
                            tap(w1, 0, [[64, 8], [1, 64]]),
                            tap(yc, 0, [[128, 8], [1, 64]]),
                            tap(yc, 64, [[128, 8], [1, 64]]), ALU.max)
                        w2 = t2p.tile([128, 256], F16)
                        nc.vector.tensor_tensor(
                            tap(w2, 0, [[32, 8], [1, 32]]),
                            tap(w1, 0, [[64, 8], [1, 32]]),
                            tap(w1, 32, [[64, 8], [1, 32]]), ALU.max)
                        nc.vector.tensor_tensor(
                            tap(mm, 128 * (2 * cc + h), [[16, 8], [1, 16]]),
                            tap(w2, 0, [[32, 8], [1, 16]]),
                            tap(w2, 16, [[32, 8], [1, 16]]), ALU.max)
                    else:
                        # route A: DVE segmented max straight out of PSUM;
                        # level members at stride G (innermost dim u)
                        nc.vector.tensor_reduce(
                            m2.rearrange("p (b g) -> p b g", g=G),
                            tap(gr, 0, [[128, 8], [1, G], [G, SEG]]),
                            axis=AX.X, op=ALU.max)
            # stage2: sqrt, sub rbar, segmented max over levels
            s2 = s2p.tile([128, 2048], F16)
            nc.scalar.activation(s2[:, :], mm[:, :], AF.Sqrt,
                                 bias=epsb[0:128, 0:1], scale=-1.0)
            y2 = y2p.tile([128, 2048], F16)
            nc.gpsimd.tensor_tensor(
                y2[:, :].rearrange("p (b g) -> p b g", g=G),
                cap(rbar, 0, [[0, 128], [1, G]]),
                s2[:, :].rearrange("p (b g) -> p b g", g=G), ALU.subtract)
            z = zpool.tile([128, 128], F16)
            nc.vector.tensor_reduce(
                z[:, :], y2[:, :].rearrange("p (b g) -> p b g", g=G),
                axis=AX.X, op=ALU.max)
            # tail for this tile
            z2 = z2pool.tile([128, 128], F16)
            nc.vector.tensor_scalar_add(z2[:, :], z[:, :],
                                        rcolp[0:128, 0:1])
            ztr = prepool.tile([128, 128], F16, tag="pre")
            nc.tensor.transpose(ztr[:, :], z2[:, :], p16[0:128, 0:128])
            osb = outp.tile([128, L], F32)
            nc.vector.tensor_reduce(
                osb[:, :], ztr[:, :].rearrange("p (a b) -> p a b", a=L),
                axis=AX.X, op=ALU.max)
            nc.sync.dma_start(out_d[128 * t:128 * (t + 1), :], osb[:, :])

    nc.compile()
    return nc


def get_nc(nt):
    key = ("nc", nt)
    if key not in _CACHE:
        _CACHE[key] = _build_nc(nt)
    return _CACHE[key]


# ---------------------------------------------------------------- entry point
def kernel(q, fixed_rot, fixed_trans, joint_axes, link_spheres,
           collision_mask):
    from concourse.bass_utils import run_bass_kernel_spmd

    q = np.asarray(q, np.float32)
    bc = B // NCORES
    nt = bc // 128
    consts = _host_consts(fixed_rot, fixed_trans, joint_axes, link_spheres,
                          collision_mask, bc)
    nc = get_nc(nt)
    in_maps = []
    for c in range(NCORES):
        m = {"q": np.ascontiguousarray(q[c * bc:(c + 1) * bc]), **consts}
        in_maps.append(m)
    res = run_bass_kernel_spmd(nc, in_maps, list(range(NCORES)))
    out = np.concatenate([res.results[c]["out"] for c in range(NCORES)],
                         axis=0)
    return out.astype(np.float32)
